# revision 46
# baseline (speedup 1.0000x reference)
"""MultiHeadedAttention Trainium2 kernel (8 NeuronCores, SPMD).

Reference computation (B=4, LQ=1024, D=1024, HEAD=16, D_K=64, H_W=1024):
    q = query; for i in 4: q = q @ Wq[i] + bq[i]           # (B, LQ, D)
    k = (key @ Wk + bk).reshape(B, HEAD, D_K, H_W)
    v = (value @ Wv + bv).reshape(B, HEAD, D_K, H_W)
    s = einsum("bhqd,bhdw->bhqw", q_heads, k) / 8
    p = softmax(s, axis=-1)            # mask is all-ones -> no-op
    x = einsum("bhqw,bhdw->bhqd", p, v)
    out = x.reshape(B, LQ, D) @ Wq[3] + bq[3]

Sharding: core c handles (b = c//2, LQ half = c%2) -> 512 query rows of one
batch, all 16 heads.  No cross-core communication.  Weights replicated.
Activations kept transposed (feature dim on partitions).

Structure (the attention exp saturates the ACT engine at ~55-70us; the
kernel hides it under PE GEMM work):
  prefix:  q-linears 0-3 (fp8 DoubleRow, 2x PE rate), v-linear rh=0 half
  pipeline per head-pair hc: k-linear chunk (fp8 DR) -> packed K=64 score
    matmuls (tile_position row-split, both heads concurrent) -> exp
    ([128,1024] ACT calls, 2 PSUM banks) -> attn@v (fp16, ones-column
    denominator) -> normalize (per-pair [2,512] DVE reciprocal + gpsimd
    broadcast).  v-linear rh=1 chunks fused into stages 0-3.
  tail: out-projection (fp16) + output DMA.

fp8 scaling convention: all fp8 weights are stored x16 (0.02-scale weights
underflow fp8 min-normal 2^-6 otherwise).  q-linear drains rescale by 1/16
(ACT identity, scale+bias fused).  q4 and kk stay x16 in fp16; the combined
x256 is folded into the exp scale (0.125/256).
"""

import numpy as np

import concourse.bass as bass
import concourse.mybir as mybir
import concourse.tile as tile
from concourse import bacc

P = 128
NCH = 8          # 1024 / 128 channel chunks
LQH = 512        # LQ rows per core
D = 1024
HEADS = 16
PAIRS = 8
DK = 64
B = 4
LQ = 1024

F32 = mybir.dt.float32
F16 = mybir.dt.float16
F8 = mybir.dt.float8e4
EXP = mybir.ActivationFunctionType.Exp
IDN = mybir.ActivationFunctionType.Identity
RCP = mybir.ActivationFunctionType.Reciprocal
DR = mybir.MatmulPerfMode.DoubleRow

WSCALE = 16.0            # fp8 weight pre-scale (host side)
EXP_SCALE = 0.125 / (WSCALE * WSCALE)   # 1/sqrt(dk) / (16*16)

# Debug taps: when non-empty, intermediate tiles are DMA'd to extra DRAM
# outputs of these names.  Dev-only; empty for the graded build.
DEBUG_TAPS = ()
DEBUG_PAIR = 0


def _emit(tc: tile.TileContext, io: dict):
    nc = tc.nc

    qT_d = io["qT"][:]
    keyT_d = io["keyT"][:]
    valueT_d = io["valueT"][:]
    wqp_d = io["Wqp"][:]      # (4, 8, 128, 8, 128) fp8 x16 col-chunks
    w3p_d = io["W3p"][:]      # (8, 128, 8, 128) fp16 col-chunks (true scale)
    wk_d = io["Wkp"][:]       # (128, 8, 1024) fp8 x16
    wv_d = io["Wvp"][:]       # (8, 128, 8, 128) fp16 col-chunks
    bq_d = io["bq"][:]        # (128, 4, 8) f32 true
    bq3x16_d = io["bq3x16"][:]  # (128, 8) f32: 16*bq[3]
    bk16_d = io["bk16"][:]    # (1024,) f32: 16*bk
    bv_d = io["bv"][:]        # (128, 8) f32
    outT_d = io["outT"][:]

    def tap(name, ap):
        if name in DEBUG_TAPS:
            nc.sync.dma_start(out=io[f"dbg_{name}"][:], in_=ap)

    with (
        tc.tile_pool(name="constp", bufs=1) as constp,
        tc.tile_pool(name="apool", bufs=1) as apool,
        tc.tile_pool(name="wpool", bufs=1) as wpool,
        tc.tile_pool(name="npool", bufs=1) as npool,
        tc.tile_pool(name="pp", bufs=1, space="PSUM") as pp,
    ):
        # ---- input + weight DMAs for the q-linear chain ------------------
        # qT on the vector queue, weights on sync: two independent rings so
        # the weight stream does not sit behind the 0.5MB activation pull.
        # vt/kt/wk (4MB) are deliberately NOT issued here -- at startup they
        # would steal HBM bandwidth from the critical linear-0 feed.
        a0 = apool.tile([P, NCH, LQH], F8, tag="a", bufs=3, name="a0")
        for cq in range(0, NCH, 4):
            nc.scalar.dma_start(
                out=a0[:, cq : cq + 4, :], in_=qT_d[:, cq : cq + 4, :]
            )
        # weight DMAs alternate sync/gpsimd rings: one ring's ~150GB/s
        # (incl. per-descriptor issue cost) cannot feed the DR-rate
        # q-linears alone.
        def wq_dma(w, src, co):
            (nc.sync if co % 2 == 0 else nc.gpsimd).dma_start(out=w, in_=src)

        wq0_pre = {}
        for co in range(NCH):
            w = wpool.tile([P, NCH, P], F8, tag="wq", bufs=12, name=f"wq0_{co}")
            wq_dma(w, wqp_d[0, co], co)
            wq0_pre[co] = w

        # ---- constants ---------------------------------------------------
        # bqs (needed by linear-0 drains) rides the gpsimd ring first; the
        # rest go on the scalar ring behind qT -- the 512KB broadcast
        # expansion of bkb would otherwise block the weight ring for ~3us.
        bqs = constp.tile([P, 4, NCH], F32, tag="bqs")
        nc.gpsimd.dma_start(out=bqs, in_=bq_d)
        bq3s = constp.tile([P, NCH], F32, tag="bq3s")
        nc.scalar.dma_start(out=bq3s, in_=bq3x16_d)
        bvs = constp.tile([P, NCH], F32, tag="bvs")
        nc.scalar.dma_start(out=bvs, in_=bv_d)
        # bk16 broadcast across partitions: bkb[p, w] = 16*bk[w]
        bkb = constp.tile([P, D], F32, tag="bkb")
        nc.scalar.dma_start(
            out=bkb, in_=bass.AP(bk16_d.tensor, 0, [[0, P], [1, D]])
        )

        # warm the ACT exp table before the pipeline needs it
        expw = constp.tile([1, 8], F32, tag="expw")
        nc.vector.memset(expw, 0.0)
        nc.scalar.activation(out=expw, in_=expw, func=EXP)

        # ---- phase 1: q-linears 0..2 (fp8 DoubleRow) --------------------
        acts = a0
        for i in range(3):
            nxt = apool.tile([P, NCH, LQH], F8, tag="a", bufs=3, name=f"a{i + 1}")
            if i == 1:
                # value^T fp16: needed by phase 2 (~30us in); the scalar
                # ring is idle after qT, and issuing here keeps the big
                # activation pulls off the startup HBM burst.
                vt = apool.tile([P, NCH, D], F16, tag="vt", name="vt")
                nc.scalar.dma_start(out=vt[:, 0:4, :], in_=valueT_d[:, 0:4, :])
                nc.scalar.dma_start(out=vt[:, 4:8, :], in_=valueT_d[:, 4:8, :])
            if i == 2:
                # key^T + Wk fp8: needed by the early-attention block right
                # after linear 3 -- ride the two weight rings (which are
                # nearly drained by now), NOT behind vt on the scalar ring.
                kt = apool.tile([P, NCH, D], F8, tag="kt", name="kt")
                nc.sync.dma_start(out=kt, in_=keyT_d)
                wk = wpool.tile([P, NCH, D], F8, tag="wk", name="wk")
                nc.gpsimd.dma_start(out=wk, in_=wk_d)
            for co in range(NCH):
                if i == 0:
                    wq_cc = wq0_pre[co]
                else:
                    wq_cc = wpool.tile(
                        [P, NCH, P], F8, tag="wq", bufs=12, name=f"wq{i}_{co}"
                    )
                    wq_dma(wq_cc, wqp_d[i, co], co)
                ps = pp.tile([P, LQH], F32, tag="px", bufs=2, name=f"psq{i}_{co}")
                for jp in range(NCH // 2):
                    nc.tensor.matmul(
                        ps,
                        lhsT=wq_cc[:, 2 * jp : 2 * jp + 2, :],
                        rhs=acts[:, 2 * jp : 2 * jp + 2, :],
                        start=(jp == 0),
                        stop=(jp == NCH // 2 - 1),
                        perf_mode=DR,
                    )
                # next act (true scale, fp8) = ps/16 + bq[i].  On DVE: the
                # ACT engine is the startup bottleneck (it issues the
                # qT/const DMAs), while the DVE is idle until phase 2.
                nc.vector.tensor_scalar(
                    out=nxt[:, co, :], in0=ps,
                    scalar1=1.0 / WSCALE, scalar2=bqs[:, i, co : co + 1],
                    op0=mybir.AluOpType.mult, op1=mybir.AluOpType.add,
                )
            acts = nxt
            if i == 0:
                tap("a1", nxt)

        # ---- phase 1b: q-linear 3 -> q4T fp16, stored x16 ---------------
        q4T = apool.tile([P, NCH, LQH], F16, tag="q4", name="q4T")
        for co in range(NCH):
            wq_cc = wpool.tile([P, NCH, P], F8, tag="wq", bufs=12, name=f"wq3_{co}")
            wq_dma(wq_cc, wqp_d[3, co], co)
            ps = pp.tile([P, LQH], F32, tag="px", bufs=2, name=f"psq3_{co}")
            for jp in range(NCH // 2):
                nc.tensor.matmul(
                    ps,
                    lhsT=wq_cc[:, 2 * jp : 2 * jp + 2, :],
                    rhs=acts[:, 2 * jp : 2 * jp + 2, :],
                    start=(jp == 0),
                    stop=(jp == NCH // 2 - 1),
                    perf_mode=DR,
                )
            # q4T = 16*q4 = ps + 16*bq3  (x16 folded into EXP_SCALE)
            nc.vector.tensor_scalar_add(
                out=q4T[:, co, :], in0=ps, scalar1=bq3s[:, co : co + 1]
            )
        tap("q4T", q4T)

        # ---- v-linear weights + out-proj weights (sync queue) -----------
        wvr = []
        for wc in range(NCH):
            w = wpool.tile([P, NCH, P], F16, tag="wv", bufs=NCH, name=f"wv{wc}")
            nc.sync.dma_start(out=w, in_=wv_d[wc])
            wvr.append(w)
        w3r = []
        for co in range(NCH):
            w = wpool.tile([P, NCH, P], F16, tag="w3", bufs=NCH, name=f"w3_{co}")
            nc.sync.dma_start(out=w, in_=w3p_d[co])
            w3r.append(w)

        # vv^T with a ones column per head: vvT4[p, wc, h, 0:64] = v^T,
        # [..., 64] = 1 -> attn@v psum row 64 = softmax denominator.
        vvT = apool.tile([P, NCH, HEADS * 65], F16, tag="vv", name="vvT")
        vvT4 = vvT.rearrange("p c (h e) -> p c h e", e=65)
        nc.vector.memset(vvT4[:, :, :, 64], 1.0)

        def v_chunk(wc, rh):
            ps = pp.tile([P, LQH], F32, tag="pk", bufs=2, name=f"psv{wc}_{rh}")
            for j in range(NCH):
                nc.tensor.matmul(
                    ps,
                    lhsT=wvr[wc][:, j, :],
                    rhs=vt[:, j, rh * LQH : (rh + 1) * LQH],
                    start=(j == 0),
                    stop=(j == NCH - 1),
                )
            nc.vector.tensor_scalar_add(
                out=vvT4[:, wc, rh * 8 : (rh + 1) * 8, 0:64],
                in0=ps.rearrange("p (h e) -> p h e", e=64),
                scalar1=bvs[:, wc : wc + 1],
            )

        # ---- pipeline helpers ------------------------------------------
        xT = apool.tile([P, NCH, LQH], F16, tag="xt", name="xT")
        pts, psxs, kks = {}, {}, {}

        def kk_block(hc, tag, single=False):
            # k-linear chunk hc: kkc = 16*k rows [hc*128, hc*128+128)
            kkc = apool.tile([P, D], F16, tag="kk", bufs=3, name=f"kk{hc}")
            psk = [None, None]
            if single:
                psk[0] = psk[1] = pp.tile(
                    [P, LQH], F32, tag=tag, bufs=2, name=f"psk{hc}"
                )
            else:
                psk = [
                    pp.tile([P, LQH], F32, tag=tag, bufs=2, name=f"psk{hc}_{wh}")
                    for wh in range(2)
                ]
            for wh in range(2):
                for jp in range(NCH // 2):
                    nc.tensor.matmul(
                        psk[wh],
                        lhsT=kt[:, 2 * jp : 2 * jp + 2, hc * P : (hc + 1) * P],
                        rhs=wk[:, 2 * jp : 2 * jp + 2, wh * LQH : (wh + 1) * LQH],
                        start=(jp == 0),
                        stop=(jp == NCH // 2 - 1),
                        perf_mode=DR,
                    )
                nc.vector.tensor_add(
                    out=kkc[:, wh * LQH : (wh + 1) * LQH],
                    in0=psk[wh],
                    in1=bkb[:, wh * LQH : (wh + 1) * LQH],
                )
            if hc == DEBUG_PAIR:
                tap("kk0", kkc)
            kks[hc] = kkc

        def new_pair(hc, pxtag):
            pts[hc] = apool.tile(
                [P, NCH, 2, LQH], F16, tag="pt", bufs=3, name=f"pt{hc}"
            )
            psxs[hc] = [
                pp.tile([P, LQH], F32, tag=pxtag, bufs=2, name=f"psx{hc}_{e}")
                for e in range(2)
            ]

        def score_block(hc, wc):
            kkc = kks[hc]
            ps2 = pp.tile([P, 2, LQH], F32, tag="p2", bufs=2, name=f"pss{hc}_{wc}")
            for e in range(2):
                nc.tensor.matmul(
                    ps2[:, e, :],
                    lhsT=kkc[e * DK : (e + 1) * DK, wc * P : (wc + 1) * P],
                    rhs=q4T[e * DK : (e + 1) * DK, hc, :],
                    start=True,
                    stop=True,
                    tile_position=(e * DK, 0),
                )
            nc.scalar.activation(
                out=pts[hc][:, wc], in_=ps2, func=EXP, scale=EXP_SCALE
            )

        def attnv(hp, wc):
            for e in range(2):
                nc.tensor.matmul(
                    psxs[hp][e][0:65, :],
                    lhsT=vvT4[:, wc, 2 * hp + e, :],
                    rhs=pts[hp][:, wc, e, :],
                    start=(wc == 0),
                    stop=(wc == NCH - 1),
                )

        def norm(hp):
            # normalize pair hp.  The DVE reciprocal is an iterative 8-cyc
            # op whose cost scales with FREE-dim elements, so a [1,512]
            # denominator row costs 3.4us.  Round-trip the rows through a
            # [128, 8] layout (two tiny sync-ring SBUF DMAs, q = 4p+g) and
            # the reciprocal drops to ~0.15us -- the pipeline is otherwise
            # DVE-bound.
            xu2 = npool.tile([P, LQH], F32, tag="xu", bufs=2, name=f"xu{hp}")
            drT = npool.tile([P, 8], F32, tag="drT", bufs=2, name=f"drT{hp}")
            dr2 = npool.tile([1, 2, LQH], F32, tag="dr2", bufs=2, name=f"dr2_{hp}")
            for e in range(2):
                nc.vector.tensor_copy(
                    xu2[e * DK : (e + 1) * DK, :], psxs[hp][e][0:DK, :]
                )
                nc.vector.tensor_copy(dr2[:, e, :], psxs[hp][e][DK : DK + 1, :])
                nc.sync.dma_start(
                    out=drT[:, 4 * e : 4 * e + 4],
                    in_=dr2[:, e, :].rearrange("o (p g) -> o p g", g=4),
                )
            rcT = npool.tile([P, 8], F32, tag="rcT", bufs=2, name=f"rcT{hp}")
            nc.vector.reciprocal(rcT, drT)
            rr2 = npool.tile([1, 2, LQH], F32, tag="rr2", bufs=2, name=f"rr2_{hp}")
            for e in range(2):
                nc.gpsimd.dma_start(
                    out=rr2[:, e, :].rearrange("o (p g) -> o p g", g=4),
                    in_=rcT[:, 4 * e : 4 * e + 4],
                )
            # broadcast both heads' reciprocal rows across their partition
            # halves with stride-0-source DMAs (gpsimd partition_broadcast
            # ignores the out base partition, and runs serially on the
            # engine; the DMA rings are idle here).
            bc2 = npool.tile([P, LQH], F32, tag="bc2", bufs=2, name=f"bc2_{hp}")
            for e in range(2):
                nc.sync.dma_start(
                    out=bc2[e * DK : (e + 1) * DK, :],
                    in_=rr2[:, e, :].unsqueeze(1).broadcast_to([1, DK, LQH]),
                )
            nc.vector.tensor_mul(out=xT[:, hp, :], in0=xu2, in1=bc2)
            if hp == DEBUG_PAIR:
                tap("pt0", pts[hp])
                tap("xu0", xu2)
                tap("bc0", bc2)

            psxs.pop(hp)
            pts.pop(hp)

        # ---- phase 2: v-linear fused with pairs 0-1 scores/exp ----------
        # ACT is the pipeline bottleneck (the softmax exp floor); two
        # pairs' worth of exp hides under the v-linear's PE work here, so
        # the steady-state pipeline below runs 6 ACT-bound stages, not 8.
        kk_block(0, "px")
        kk_block(1, "px")
        # kk(2)/kk(3) early in single-psum-slot mode: stages 2-3 then have
        # no k-linear, so their scores/exp can start as soon as the p2/pt
        # slots rotate -- ACT runs exp continuously from here on.
        kk_block(2, "px", single=True)
        kk_block(3, "px", single=True)
        new_pair(0, "px")
        new_pair(1, "px")
        for wc in range(NCH):
            v_chunk(wc, 0)
            score_block(0, wc)
        for wc in range(NCH):
            v_chunk(wc, 1)
            score_block(1, wc)
            attnv(0, wc)
        norm(0)

        # ---- phase 3: steady-state pipeline over pairs 2..7 -------------
        for hc in range(2, PAIRS):
            if hc >= 4:
                kk_block(hc, "pk")
            # the last pair's attn@v runs inside its own stage (2-chunk
            # lag behind exp) and borrows the freed kk psum slots so it
            # does not contend with pair 6's drains.
            new_pair(hc, "pk" if hc == PAIRS - 1 else "px")
            for wc in range(NCH):
                score_block(hc, wc)
                attnv(hc - 1, wc)
                if hc == PAIRS - 1 and wc >= 2:
                    attnv(hc, wc - 2)
            if hc == PAIRS - 1:
                attnv(hc, NCH - 2)
                attnv(hc, NCH - 1)
            norm(hc - 1)
            if hc == PAIRS - 1:
                norm(hc)

        tap("vvT", vvT)
        tap("xT", xT)

        # ---- phase 4: out projection (fp16, reuses Wq[3]/bq[3]) ---------
        # Six psum accumulators in flight (cycling the px/pk/p2 slots that
        # the pipeline has released) so every chunk's j<=6 matmuls run
        # during the final normalize chains; only the j=7 matmul of each
        # chunk waits on the last pair.  Output DMAs fan across 3 rings.
        outq = [nc.scalar, nc.gpsimd, nc.sync]
        for co in range(NCH):
            t = co % 3
            if t == 2:
                ps2t = pp.tile([P, 2, LQH], F32, tag="p2", bufs=2, name=f"pso{co}")
                ps = ps2t[:, 0, :]
            else:
                ps = pp.tile(
                    [P, LQH], F32, tag=("px", "pk")[t], bufs=2, name=f"pso{co}"
                )
            for j in range(NCH):
                nc.tensor.matmul(
                    ps,
                    lhsT=w3r[co][:, j, :],
                    rhs=xT[:, j, :],
                    start=(j == 0),
                    stop=(j == NCH - 1),
                )
            outsb = npool.tile([P, LQH], F32, tag="osb", bufs=2, name=f"osb{co}")
            nc.vector.tensor_scalar_add(
                out=outsb, in0=ps, scalar1=bqs[:, 3, co : co + 1]
            )
            outq[co % 3].dma_start(
                out=outT_d.rearrange("(c p) q -> p c q", p=P)[:, co, :],
                in_=outsb,
            )


def build_nc():
    nc = bacc.Bacc("TRN2", target_bir_lowering=False)
    io = {}
    io["qT"] = nc.dram_tensor("qT", [P, NCH, LQH], F8, kind="ExternalInput")
    io["keyT"] = nc.dram_tensor("keyT", [P, NCH, D], F8, kind="ExternalInput")
    io["valueT"] = nc.dram_tensor("valueT", [P, NCH, D], F16, kind="ExternalInput")
    io["Wqp"] = nc.dram_tensor("Wqp", [4, NCH, P, NCH, P], F8, kind="ExternalInput")
    io["W3p"] = nc.dram_tensor("W3p", [NCH, P, NCH, P], F16, kind="ExternalInput")
    io["Wkp"] = nc.dram_tensor("Wkp", [P, NCH, D], F8, kind="ExternalInput")
    io["Wvp"] = nc.dram_tensor("Wvp", [NCH, P, NCH, P], F16, kind="ExternalInput")
    io["bq"] = nc.dram_tensor("bq", [P, 4, NCH], F32, kind="ExternalInput")
    io["bq3x16"] = nc.dram_tensor("bq3x16", [P, NCH], F32, kind="ExternalInput")
    io["bk16"] = nc.dram_tensor("bk16", [D], F32, kind="ExternalInput")
    io["bv"] = nc.dram_tensor("bv", [P, NCH], F32, kind="ExternalInput")
    io["outT"] = nc.dram_tensor("outT", [D, LQH], F32, kind="ExternalOutput")
    _dbg_shapes = {
        "a1": ([P, NCH, LQH], F8),
        "q4T": ([P, NCH, LQH], F16),
        "kk0": ([P, D], F16),
        "pt0": ([P, NCH, 2, LQH], F16),
        "xu0": ([P, LQH], F32),
        "bc0": ([P, LQH], F32),
        "rb0": ([P, LQH], F32),
        "vvT": ([P, NCH, HEADS * 65], F16),
        "xT": ([P, NCH, LQH], F16),
    }
    for name in DEBUG_TAPS:
        shape, dt_ = _dbg_shapes[name]
        io[f"dbg_{name}"] = nc.dram_tensor(f"dbg_{name}", shape, dt_, kind="ExternalOutput")
    with tile.TileContext(nc) as tc:
        _emit(tc, io)
    nc.finalize()
    return nc


def _np_f8():
    import ml_dtypes

    return np.dtype(ml_dtypes.float8_e4m3)


def _pack_wq(Wq: np.ndarray):
    # [i, j*128+p, co*128+n] -> [i, co, p, j, n]
    A = Wq.reshape(4, NCH, P, NCH, P).transpose(0, 3, 2, 1, 4)
    wqp = np.ascontiguousarray(A * WSCALE).astype(_np_f8())
    w3p = np.ascontiguousarray(A[3]).astype(np.float16)
    return wqp, w3p


def _pack_wv(Wv: np.ndarray) -> np.ndarray:
    A = Wv.reshape(NCH, P, NCH, P)             # [j, p, co, n]
    return np.ascontiguousarray(A.transpose(2, 1, 0, 3)).astype(np.float16)


def _pack_wk(Wk: np.ndarray) -> np.ndarray:
    # [j*128+p, w] -> [p, j, w], x16 fp8
    A = (Wk * WSCALE).reshape(NCH, P, D).transpose(1, 0, 2)
    return np.ascontiguousarray(A).astype(_np_f8())


def make_in_maps(query, key, value, Wq, bq, Wk, bk, Wv, bv):
    f8 = _np_f8()
    Wqp, W3p = _pack_wq(Wq)
    Wvp = _pack_wv(Wv)
    Wkp = _pack_wk(Wk)
    bqp = np.ascontiguousarray(bq.reshape(4, NCH, P).transpose(2, 0, 1))
    bq3x16 = np.ascontiguousarray((bq[3] * WSCALE).reshape(NCH, P).T)
    bvp = np.ascontiguousarray(bv.reshape(NCH, P).T)
    bk16 = np.ascontiguousarray(bk * WSCALE)
    in_maps = []
    for c in range(8):
        b, half = c // 2, c % 2
        in_maps.append(
            {
                "qT": np.ascontiguousarray(
                    query[b, half * LQH : (half + 1) * LQH, :].T
                    .reshape(NCH, P, LQH).transpose(1, 0, 2)
                ).astype(f8),
                "keyT": np.ascontiguousarray(
                    key[b].T.reshape(NCH, P, D).transpose(1, 0, 2)
                ).astype(f8),
                "valueT": np.ascontiguousarray(
                    value[b].T.reshape(NCH, P, D).transpose(1, 0, 2)
                ).astype(np.float16),
                "Wqp": Wqp,
                "W3p": W3p,
                "Wkp": Wkp,
                "Wvp": Wvp,
                "bq": bqp,
                "bq3x16": bq3x16,
                "bk16": bk16,
                "bv": bvp,
            }
        )
    return in_maps


_NC_CACHE = None


def _get_nc():
    global _NC_CACHE
    if _NC_CACHE is None:
        _NC_CACHE = build_nc()
    return _NC_CACHE


def _numpy_fallback(query, key, value, mask, Wq, bq, Wk, bk, Wv, bv):
    q = query.astype(np.float64)
    for i in range(4):
        q = q @ Wq[i] + bq[i]
    q = q.reshape(B, LQ, HEADS, DK).transpose(0, 2, 1, 3)
    k = (key @ Wk + bk).reshape(B, HEADS, DK, D)
    v = (value @ Wv + bv).reshape(B, HEADS, DK, D)
    s = np.einsum("bhqd,bhdw->bhqw", q, k) / np.sqrt(DK)
    s = np.where(mask[:, None, :, :] == 0, -1e9, s)
    s = s - s.max(axis=-1, keepdims=True)
    p = np.exp(s)
    p /= p.sum(axis=-1, keepdims=True)
    x = np.einsum("bhqw,bhdw->bhqd", p, v)
    x = x.transpose(0, 2, 1, 3).reshape(B, LQ, D)
    return (x @ Wq[3] + bq[3]).astype(np.float32)


def kernel(query, key, value, mask, Wq, bq, Wk, bk, Wv, bv):
    query = np.asarray(query, np.float32)
    key = np.asarray(key, np.float32)
    value = np.asarray(value, np.float32)
    mask = np.asarray(mask)
    Wq = np.asarray(Wq, np.float32)
    bq = np.asarray(bq, np.float32)
    Wk = np.asarray(Wk, np.float32)
    bk = np.asarray(bk, np.float32)
    Wv = np.asarray(Wv, np.float32)
    bv = np.asarray(bv, np.float32)

    if not mask.all():
        # Never hit with the reference generator (mask is all-ones); kept for
        # functional completeness.
        return _numpy_fallback(query, key, value, mask, Wq, bq, Wk, bk, Wv, bv)

    from concourse.bass_utils import run_bass_kernel_spmd

    nc = _get_nc()
    in_maps = make_in_maps(query, key, value, Wq, bq, Wk, bk, Wv, bv)
    res = run_bass_kernel_spmd(nc, in_maps, core_ids=list(range(8)))
    out = np.empty((B, LQ, D), np.float32)
    for c in range(8):
        b, half = c // 2, c % 2
        out[b, half * LQH : (half + 1) * LQH, :] = res.results[c]["outT"].T
    return out


# revision 48
# speedup vs baseline: 1.2535x; 1.2535x over previous
"""MultiHeadedAttention Trainium2 kernel (8 NeuronCores, SPMD).

Reference computation (B=4, LQ=1024, D=1024, HEAD=16, D_K=64, H_W=1024):
    q = query; for i in 4: q = q @ Wq[i] + bq[i]           # (B, LQ, D)
    k = (key @ Wk + bk).reshape(B, HEAD, D_K, H_W)
    v = (value @ Wv + bv).reshape(B, HEAD, D_K, H_W)
    s = einsum("bhqd,bhdw->bhqw", q_heads, k) / 8
    p = softmax(s, axis=-1)            # mask is all-ones -> no-op
    x = einsum("bhqw,bhdw->bhqd", p, v)
    out = x.reshape(B, LQ, D) @ Wq[3] + bq[3]

Sharding: core c handles (b = c//2, LQ half = c%2) -> 512 query rows of one
batch, all 16 heads.  No cross-core communication.  Weights replicated.
Activations kept transposed (feature dim on partitions).

Structure (the attention exp saturates the ACT engine at ~55-70us; the
kernel hides it under PE GEMM work):
  prefix:  q-linears 0-3 (fp8 DoubleRow, 2x PE rate), v-linear rh=0 half
  pipeline per head-pair hc: k-linear chunk (fp8 DR) -> packed K=64 score
    matmuls (tile_position row-split, both heads concurrent) -> exp
    ([128,1024] ACT calls, 2 PSUM banks) -> attn@v (fp16, ones-column
    denominator) -> normalize (per-pair [2,512] DVE reciprocal + gpsimd
    broadcast).  v-linear rh=1 chunks fused into stages 0-3.
  tail: out-projection (fp16) + output DMA.

fp8 scaling convention: all fp8 weights are stored x16 (0.02-scale weights
underflow fp8 min-normal 2^-6 otherwise).  q-linear drains rescale by 1/16
(ACT identity, scale+bias fused).  q4 and kk stay x16 in fp16; the combined
x256 is folded into the exp scale (0.125/256).
"""

import numpy as np

import concourse.bass as bass
import concourse.mybir as mybir
import concourse.tile as tile
from concourse import bacc

P = 128
NCH = 8          # 1024 / 128 channel chunks
LQH = 512        # LQ rows per core
D = 1024
HEADS = 16
PAIRS = 8
DK = 64
B = 4
LQ = 1024

F32 = mybir.dt.float32
F16 = mybir.dt.float16
F8 = mybir.dt.float8e4
EXP = mybir.ActivationFunctionType.Exp
IDN = mybir.ActivationFunctionType.Identity
RCP = mybir.ActivationFunctionType.Reciprocal
DR = mybir.MatmulPerfMode.DoubleRow

WSCALE = 16.0            # fp8 weight pre-scale (host side)
EXP_SCALE = 0.125 / (WSCALE * WSCALE)   # 1/sqrt(dk) / (16*16)

# Debug taps: when non-empty, intermediate tiles are DMA'd to extra DRAM
# outputs of these names.  Dev-only; empty for the graded build.
DEBUG_TAPS = ()
DEBUG_PAIR = 0


def _emit(tc: tile.TileContext, io: dict):
    nc = tc.nc

    qT_d = io["qT"][:]
    keyT_d = io["keyT"][:]
    valueT_d = io["valueT"][:]
    wqp_d = io["Wqp"][:]      # (4, 8, 128, 8, 128) fp8 x16 col-chunks
    w3p_d = io["W3p"][:]      # (8, 128, 8, 128) fp16 col-chunks (true scale)
    wk_d = io["Wkp"][:]       # (128, 8, 1024) fp8 x16
    wv_d = io["Wvp"][:]       # (8, 128, 8, 128) fp16 col-chunks
    bq_d = io["bq"][:]        # (128, 4, 8) f32 true
    bq3x16_d = io["bq3x16"][:]  # (128, 8) f32: 16*bq[3]
    bk16_d = io["bk16"][:]    # (1024,) f32: 16*bk
    bv_d = io["bv"][:]        # (128, 8) f32
    outT_d = io["outT"][:]

    def tap(name, ap):
        if name in DEBUG_TAPS:
            nc.sync.dma_start(out=io[f"dbg_{name}"][:], in_=ap)

    with (
        tc.tile_pool(name="constp", bufs=1) as constp,
        tc.tile_pool(name="apool", bufs=1) as apool,
        tc.tile_pool(name="wpool", bufs=1) as wpool,
        tc.tile_pool(name="npool", bufs=1) as npool,
        tc.tile_pool(name="pp", bufs=1, space="PSUM") as pp,
    ):
        # ---- input + weight DMAs for the q-linear chain ------------------
        # qT on the vector queue, weights on sync: two independent rings so
        # the weight stream does not sit behind the 0.5MB activation pull.
        # vt/kt/wk (4MB) are deliberately NOT issued here -- at startup they
        # would steal HBM bandwidth from the critical linear-0 feed.
        a0 = apool.tile([P, NCH, LQH], F8, tag="a", bufs=3, name="a0")
        for cq in range(0, NCH, 4):
            nc.scalar.dma_start(
                out=a0[:, cq : cq + 4, :], in_=qT_d[:, cq : cq + 4, :]
            )
        # weight DMAs alternate sync/gpsimd rings: one ring's ~150GB/s
        # (incl. per-descriptor issue cost) cannot feed the DR-rate
        # q-linears alone.
        def wq_dma(w, src, co):
            (nc.sync if co % 2 == 0 else nc.gpsimd).dma_start(out=w, in_=src)

        wq0_pre = {}
        for co in range(NCH):
            w = wpool.tile([P, NCH, P], F8, tag="wq", bufs=9, name=f"wq0_{co}")
            wq_dma(w, wqp_d[0, co], co)
            wq0_pre[co] = w

        # ---- constants ---------------------------------------------------
        # bqs (needed by linear-0 drains) rides the gpsimd ring first; the
        # rest go on the scalar ring behind qT -- the 512KB broadcast
        # expansion of bkb would otherwise block the weight ring for ~3us.
        bqs = constp.tile([P, 4, NCH], F32, tag="bqs")
        nc.gpsimd.dma_start(out=bqs, in_=bq_d)
        bq3s = constp.tile([P, NCH], F32, tag="bq3s")
        nc.scalar.dma_start(out=bq3s, in_=bq3x16_d)
        bvs = constp.tile([P, NCH], F32, tag="bvs")
        nc.scalar.dma_start(out=bvs, in_=bv_d)
        # bk16 broadcast across partitions: bkb[p, w] = 16*bk[w]
        bkb = constp.tile([P, D], F32, tag="bkb")
        nc.scalar.dma_start(
            out=bkb, in_=bass.AP(bk16_d.tensor, 0, [[0, P], [1, D]])
        )

        # warm the ACT exp table before the pipeline needs it
        expw = constp.tile([1, 8], F32, tag="expw")
        nc.vector.memset(expw, 0.0)
        nc.scalar.activation(out=expw, in_=expw, func=EXP)

        # ---- phase 1: q-linears 0..2 (fp8 DoubleRow) --------------------
        acts = a0
        for i in range(3):
            nxt = apool.tile([P, NCH, LQH], F8, tag="a", bufs=3, name=f"a{i + 1}")
            if i == 1:
                # value^T fp16: needed by phase 2 (~30us in); the scalar
                # ring is idle after qT, and issuing here keeps the big
                # activation pulls off the startup HBM burst.
                vt = apool.tile([P, NCH, D], F16, tag="vt", name="vt")
                nc.scalar.dma_start(out=vt[:, 0:4, :], in_=valueT_d[:, 0:4, :])
                nc.scalar.dma_start(out=vt[:, 4:8, :], in_=valueT_d[:, 4:8, :])
            if i == 2:
                # key^T + Wk fp8: needed by the early-attention block right
                # after linear 3 -- ride the two weight rings (which are
                # nearly drained by now), NOT behind vt on the scalar ring.
                kt = apool.tile([P, NCH, D], F8, tag="kt", name="kt")
                nc.sync.dma_start(out=kt, in_=keyT_d)
                wk = wpool.tile([P, NCH, D], F8, tag="wk", name="wk")
                nc.gpsimd.dma_start(out=wk, in_=wk_d)
            for co in range(NCH):
                if i == 0:
                    wq_cc = wq0_pre[co]
                else:
                    wq_cc = wpool.tile(
                        [P, NCH, P], F8, tag="wq", bufs=9, name=f"wq{i}_{co}"
                    )
                    wq_dma(wq_cc, wqp_d[i, co], co)
                ps = pp.tile([P, LQH], F32, tag="px", bufs=2, name=f"psq{i}_{co}")
                for jp in range(NCH // 2):
                    nc.tensor.matmul(
                        ps,
                        lhsT=wq_cc[:, 2 * jp : 2 * jp + 2, :],
                        rhs=acts[:, 2 * jp : 2 * jp + 2, :],
                        start=(jp == 0),
                        stop=(jp == NCH // 2 - 1),
                        perf_mode=DR,
                    )
                # next act (true scale, fp8) = ps/16 + bq[i].  On DVE: the
                # ACT engine is the startup bottleneck (it issues the
                # qT/const DMAs), while the DVE is idle until phase 2.
                nc.vector.tensor_scalar(
                    out=nxt[:, co, :], in0=ps,
                    scalar1=1.0 / WSCALE, scalar2=bqs[:, i, co : co + 1],
                    op0=mybir.AluOpType.mult, op1=mybir.AluOpType.add,
                )
            acts = nxt
            if i == 0:
                tap("a1", nxt)

        # ---- phase 1b: q-linear 3 -> q4T fp16, stored x16 ---------------
        q4T = apool.tile([P, NCH, LQH], F16, tag="q4", name="q4T")
        for co in range(NCH):
            wq_cc = wpool.tile([P, NCH, P], F8, tag="wq", bufs=9, name=f"wq3_{co}")
            wq_dma(wq_cc, wqp_d[3, co], co)
            ps = pp.tile([P, LQH], F32, tag="px", bufs=2, name=f"psq3_{co}")
            for jp in range(NCH // 2):
                nc.tensor.matmul(
                    ps,
                    lhsT=wq_cc[:, 2 * jp : 2 * jp + 2, :],
                    rhs=acts[:, 2 * jp : 2 * jp + 2, :],
                    start=(jp == 0),
                    stop=(jp == NCH // 2 - 1),
                    perf_mode=DR,
                )
            # q4T = 16*q4 = ps + 16*bq3  (x16 folded into EXP_SCALE)
            nc.vector.tensor_scalar_add(
                out=q4T[:, co, :], in0=ps, scalar1=bq3s[:, co : co + 1]
            )
        tap("q4T", q4T)

        # ---- v-linear weights + out-proj weights (sync queue) -----------
        wvr = []
        for wc in range(NCH):
            w = wpool.tile([P, NCH, P], F16, tag="wv", bufs=NCH, name=f"wv{wc}")
            nc.sync.dma_start(out=w, in_=wv_d[wc])
            wvr.append(w)
        w3r = []
        for co in range(NCH):
            w = wpool.tile([P, NCH, P], F16, tag="w3", bufs=NCH, name=f"w3_{co}")
            nc.sync.dma_start(out=w, in_=w3p_d[co])
            w3r.append(w)

        # vv^T with a ones column per head: vvT4[p, wc, h, 0:64] = v^T,
        # [..., 64] = 1 -> attn@v psum row 64 = softmax denominator.
        vvT = apool.tile([P, NCH, HEADS * 65], F16, tag="vv", name="vvT")
        vvT4 = vvT.rearrange("p c (h e) -> p c h e", e=65)
        nc.vector.memset(vvT4[:, :, :, 64], 1.0)

        def v_chunk(wc, rh):
            ps = pp.tile([P, LQH], F32, tag="pk", bufs=2, name=f"psv{wc}_{rh}")
            for j in range(NCH):
                nc.tensor.matmul(
                    ps,
                    lhsT=wvr[wc][:, j, :],
                    rhs=vt[:, j, rh * LQH : (rh + 1) * LQH],
                    start=(j == 0),
                    stop=(j == NCH - 1),
                )
            nc.vector.tensor_scalar_add(
                out=vvT4[:, wc, rh * 8 : (rh + 1) * 8, 0:64],
                in0=ps.rearrange("p (h e) -> p h e", e=64),
                scalar1=bvs[:, wc : wc + 1],
            )

        # ---- pipeline helpers ------------------------------------------
        xT = apool.tile([P, NCH, LQH], F16, tag="xt", name="xT")
        pts, psxs, kks = {}, {}, {}

        def kk_block(hc, tag, single=False):
            # k-linear chunk hc: kkc = 16*k rows [hc*128, hc*128+128)
            kkc = apool.tile([P, D], F16, tag="kk", bufs=3, name=f"kk{hc}")
            psk = [None, None]
            if single:
                psk[0] = psk[1] = pp.tile(
                    [P, LQH], F32, tag=tag, bufs=2, name=f"psk{hc}"
                )
            else:
                psk = [
                    pp.tile([P, LQH], F32, tag=tag, bufs=2, name=f"psk{hc}_{wh}")
                    for wh in range(2)
                ]
            for wh in range(2):
                for jp in range(NCH // 2):
                    nc.tensor.matmul(
                        psk[wh],
                        lhsT=kt[:, 2 * jp : 2 * jp + 2, hc * P : (hc + 1) * P],
                        rhs=wk[:, 2 * jp : 2 * jp + 2, wh * LQH : (wh + 1) * LQH],
                        start=(jp == 0),
                        stop=(jp == NCH // 2 - 1),
                        perf_mode=DR,
                    )
                nc.vector.tensor_add(
                    out=kkc[:, wh * LQH : (wh + 1) * LQH],
                    in0=psk[wh],
                    in1=bkb[:, wh * LQH : (wh + 1) * LQH],
                )
            if hc == DEBUG_PAIR:
                tap("kk0", kkc)
            kks[hc] = kkc

        def new_pair(hc, pxtag):
            pts[hc] = apool.tile(
                [P, NCH, 2, LQH], F16, tag="pt", bufs=3, name=f"pt{hc}"
            )
            psxs[hc] = [
                pp.tile([P, LQH], F32, tag=pxtag, bufs=2, name=f"psx{hc}_{e}")
                for e in range(2)
            ]

        def score_block(hc, wc):
            kkc = kks[hc]
            ps2 = pp.tile([P, 2, LQH], F32, tag="p2", bufs=2, name=f"pss{hc}_{wc}")
            for e in range(2):
                nc.tensor.matmul(
                    ps2[:, e, :],
                    lhsT=kkc[e * DK : (e + 1) * DK, wc * P : (wc + 1) * P],
                    rhs=q4T[e * DK : (e + 1) * DK, hc, :],
                    start=True,
                    stop=True,
                    tile_position=(e * DK, 0),
                )
            nc.scalar.activation(
                out=pts[hc][:, wc], in_=ps2, func=EXP, scale=EXP_SCALE
            )

        def attnv(hp, wc):
            for e in range(2):
                nc.tensor.matmul(
                    psxs[hp][e][0:65, :],
                    lhsT=vvT4[:, wc, 2 * hp + e, :],
                    rhs=pts[hp][:, wc, e, :],
                    start=(wc == 0),
                    stop=(wc == NCH - 1),
                )

        def norm(hp):
            # normalize pair hp.  The DVE reciprocal is an iterative 8-cyc
            # op whose cost scales with FREE-dim elements, so a [1,512]
            # denominator row costs 3.4us.  Round-trip the rows through a
            # [128, 8] layout (two tiny sync-ring SBUF DMAs, q = 4p+g) and
            # the reciprocal drops to ~0.15us -- the pipeline is otherwise
            # DVE-bound.
            xu2 = npool.tile([P, LQH], F32, tag="xu", bufs=2, name=f"xu{hp}")
            drT = npool.tile([P, 8], F32, tag="drT", bufs=2, name=f"drT{hp}")
            dr2 = npool.tile([1, 2, LQH], F32, tag="dr2", bufs=2, name=f"dr2_{hp}")
            for e in range(2):
                nc.vector.tensor_copy(
                    xu2[e * DK : (e + 1) * DK, :], psxs[hp][e][0:DK, :]
                )
                nc.vector.tensor_copy(dr2[:, e, :], psxs[hp][e][DK : DK + 1, :])
                nc.sync.dma_start(
                    out=drT[:, 4 * e : 4 * e + 4],
                    in_=dr2[:, e, :].rearrange("o (p g) -> o p g", g=4),
                )
            rcT = npool.tile([P, 8], F32, tag="rcT", bufs=2, name=f"rcT{hp}")
            nc.vector.reciprocal(rcT, drT)
            rr2 = npool.tile([1, 2, LQH], F32, tag="rr2", bufs=2, name=f"rr2_{hp}")
            for e in range(2):
                nc.gpsimd.dma_start(
                    out=rr2[:, e, :].rearrange("o (p g) -> o p g", g=4),
                    in_=rcT[:, 4 * e : 4 * e + 4],
                )
            # HW partition_broadcast always lands on partitions
            # 0..channels-1 (it ignores the out AP base): broadcast
            # head-even into bc2[0:64] directly; head-odd full-height
            # into a scratch tile, then base-matched copy of its upper
            # half into bc2[64:128].  (DMA-based broadcasts chained too
            # much latency into the norm and stalled the pipeline.)
            bc2 = npool.tile([P, LQH], F32, tag="bc2", bufs=2, name=f"bc2_{hp}")
            nc.gpsimd.partition_broadcast(bc2[0:DK, :], rr2[:, 0, :])
            bco = npool.tile([P, LQH], F32, tag="bco", bufs=2, name=f"bco{hp}")
            nc.gpsimd.partition_broadcast(bco, rr2[:, 1, :])
            nc.vector.tensor_copy(bc2[DK:P, :], bco[DK:P, :])
            nc.vector.tensor_mul(out=xT[:, hp, :], in0=xu2, in1=bc2)
            if hp == DEBUG_PAIR:
                tap("pt0", pts[hp])
                tap("xu0", xu2)
                tap("bc0", bc2)

            psxs.pop(hp)
            pts.pop(hp)

        # ---- phase 2: v-linear fused with pairs 0-1 scores/exp ----------
        # ACT is the pipeline bottleneck (the softmax exp floor); two
        # pairs' worth of exp hides under the v-linear's PE work here, so
        # the steady-state pipeline below runs 6 ACT-bound stages, not 8.
        kk_block(0, "px")
        kk_block(1, "px")
        # kk(2)/kk(3) early in single-psum-slot mode: stages 2-3 then have
        # no k-linear, so their scores/exp can start as soon as the p2/pt
        # slots rotate -- ACT runs exp continuously from here on.
        kk_block(2, "px", single=True)
        kk_block(3, "px", single=True)
        new_pair(0, "px")
        new_pair(1, "px")
        for wc in range(NCH):
            v_chunk(wc, 0)
            score_block(0, wc)
        for wc in range(NCH):
            v_chunk(wc, 1)
            score_block(1, wc)
            attnv(0, wc)
        norm(0)

        # ---- phase 3: steady-state pipeline over pairs 2..7 -------------
        for hc in range(2, PAIRS):
            if hc >= 4:
                kk_block(hc, "pk")
            # the last pair's attn@v runs inside its own stage (2-chunk
            # lag behind exp) and borrows the freed kk psum slots so it
            # does not contend with pair 6's drains.
            new_pair(hc, "pk" if hc == PAIRS - 1 else "px")
            for wc in range(NCH):
                score_block(hc, wc)
                attnv(hc - 1, wc)
                if hc == PAIRS - 1 and wc >= 2:
                    attnv(hc, wc - 2)
            if hc == PAIRS - 1:
                attnv(hc, NCH - 2)
                attnv(hc, NCH - 1)
            norm(hc - 1)
            if hc == PAIRS - 1:
                norm(hc)

        tap("vvT", vvT)
        tap("xT", xT)

        # ---- phase 4: out projection (fp16, reuses Wq[3]/bq[3]) ---------
        # Six psum accumulators in flight (cycling the px/pk/p2 slots that
        # the pipeline has released) so every chunk's j<=6 matmuls run
        # during the final normalize chains; only the j=7 matmul of each
        # chunk waits on the last pair.  Output DMAs fan across 3 rings.
        outq = [nc.scalar, nc.gpsimd, nc.sync]
        for co in range(NCH):
            t = co % 3
            if t == 2:
                ps2t = pp.tile([P, 2, LQH], F32, tag="p2", bufs=2, name=f"pso{co}")
                ps = ps2t[:, 0, :]
            else:
                ps = pp.tile(
                    [P, LQH], F32, tag=("px", "pk")[t], bufs=2, name=f"pso{co}"
                )
            for j in range(NCH):
                nc.tensor.matmul(
                    ps,
                    lhsT=w3r[co][:, j, :],
                    rhs=xT[:, j, :],
                    start=(j == 0),
                    stop=(j == NCH - 1),
                )
            outsb = npool.tile([P, LQH], F32, tag="osb", bufs=2, name=f"osb{co}")
            nc.vector.tensor_scalar_add(
                out=outsb, in0=ps, scalar1=bqs[:, 3, co : co + 1]
            )
            outq[co % 3].dma_start(
                out=outT_d.rearrange("(c p) q -> p c q", p=P)[:, co, :],
                in_=outsb,
            )


def build_nc():
    nc = bacc.Bacc("TRN2", target_bir_lowering=False)
    io = {}
    io["qT"] = nc.dram_tensor("qT", [P, NCH, LQH], F8, kind="ExternalInput")
    io["keyT"] = nc.dram_tensor("keyT", [P, NCH, D], F8, kind="ExternalInput")
    io["valueT"] = nc.dram_tensor("valueT", [P, NCH, D], F16, kind="ExternalInput")
    io["Wqp"] = nc.dram_tensor("Wqp", [4, NCH, P, NCH, P], F8, kind="ExternalInput")
    io["W3p"] = nc.dram_tensor("W3p", [NCH, P, NCH, P], F16, kind="ExternalInput")
    io["Wkp"] = nc.dram_tensor("Wkp", [P, NCH, D], F8, kind="ExternalInput")
    io["Wvp"] = nc.dram_tensor("Wvp", [NCH, P, NCH, P], F16, kind="ExternalInput")
    io["bq"] = nc.dram_tensor("bq", [P, 4, NCH], F32, kind="ExternalInput")
    io["bq3x16"] = nc.dram_tensor("bq3x16", [P, NCH], F32, kind="ExternalInput")
    io["bk16"] = nc.dram_tensor("bk16", [D], F32, kind="ExternalInput")
    io["bv"] = nc.dram_tensor("bv", [P, NCH], F32, kind="ExternalInput")
    io["outT"] = nc.dram_tensor("outT", [D, LQH], F32, kind="ExternalOutput")
    _dbg_shapes = {
        "a1": ([P, NCH, LQH], F8),
        "q4T": ([P, NCH, LQH], F16),
        "kk0": ([P, D], F16),
        "pt0": ([P, NCH, 2, LQH], F16),
        "xu0": ([P, LQH], F32),
        "bc0": ([P, LQH], F32),
        "rb0": ([P, LQH], F32),
        "vvT": ([P, NCH, HEADS * 65], F16),
        "xT": ([P, NCH, LQH], F16),
    }
    for name in DEBUG_TAPS:
        shape, dt_ = _dbg_shapes[name]
        io[f"dbg_{name}"] = nc.dram_tensor(f"dbg_{name}", shape, dt_, kind="ExternalOutput")
    with tile.TileContext(nc) as tc:
        _emit(tc, io)
    nc.finalize()
    return nc


def _np_f8():
    import ml_dtypes

    return np.dtype(ml_dtypes.float8_e4m3)


def _pack_wq(Wq: np.ndarray):
    # [i, j*128+p, co*128+n] -> [i, co, p, j, n]
    A = Wq.reshape(4, NCH, P, NCH, P).transpose(0, 3, 2, 1, 4)
    wqp = np.ascontiguousarray(A * WSCALE).astype(_np_f8())
    w3p = np.ascontiguousarray(A[3]).astype(np.float16)
    return wqp, w3p


def _pack_wv(Wv: np.ndarray) -> np.ndarray:
    A = Wv.reshape(NCH, P, NCH, P)             # [j, p, co, n]
    return np.ascontiguousarray(A.transpose(2, 1, 0, 3)).astype(np.float16)


def _pack_wk(Wk: np.ndarray) -> np.ndarray:
    # [j*128+p, w] -> [p, j, w], x16 fp8
    A = (Wk * WSCALE).reshape(NCH, P, D).transpose(1, 0, 2)
    return np.ascontiguousarray(A).astype(_np_f8())


def make_in_maps(query, key, value, Wq, bq, Wk, bk, Wv, bv):
    f8 = _np_f8()
    Wqp, W3p = _pack_wq(Wq)
    Wvp = _pack_wv(Wv)
    Wkp = _pack_wk(Wk)
    bqp = np.ascontiguousarray(bq.reshape(4, NCH, P).transpose(2, 0, 1))
    bq3x16 = np.ascontiguousarray((bq[3] * WSCALE).reshape(NCH, P).T)
    bvp = np.ascontiguousarray(bv.reshape(NCH, P).T)
    bk16 = np.ascontiguousarray(bk * WSCALE)
    in_maps = []
    for c in range(8):
        b, half = c // 2, c % 2
        in_maps.append(
            {
                "qT": np.ascontiguousarray(
                    query[b, half * LQH : (half + 1) * LQH, :].T
                    .reshape(NCH, P, LQH).transpose(1, 0, 2)
                ).astype(f8),
                "keyT": np.ascontiguousarray(
                    key[b].T.reshape(NCH, P, D).transpose(1, 0, 2)
                ).astype(f8),
                "valueT": np.ascontiguousarray(
                    value[b].T.reshape(NCH, P, D).transpose(1, 0, 2)
                ).astype(np.float16),
                "Wqp": Wqp,
                "W3p": W3p,
                "Wkp": Wkp,
                "Wvp": Wvp,
                "bq": bqp,
                "bq3x16": bq3x16,
                "bk16": bk16,
                "bv": bvp,
            }
        )
    return in_maps


_NC_CACHE = None


def _get_nc():
    global _NC_CACHE
    if _NC_CACHE is None:
        _NC_CACHE = build_nc()
    return _NC_CACHE


def _numpy_fallback(query, key, value, mask, Wq, bq, Wk, bk, Wv, bv):
    q = query.astype(np.float64)
    for i in range(4):
        q = q @ Wq[i] + bq[i]
    q = q.reshape(B, LQ, HEADS, DK).transpose(0, 2, 1, 3)
    k = (key @ Wk + bk).reshape(B, HEADS, DK, D)
    v = (value @ Wv + bv).reshape(B, HEADS, DK, D)
    s = np.einsum("bhqd,bhdw->bhqw", q, k) / np.sqrt(DK)
    s = np.where(mask[:, None, :, :] == 0, -1e9, s)
    s = s - s.max(axis=-1, keepdims=True)
    p = np.exp(s)
    p /= p.sum(axis=-1, keepdims=True)
    x = np.einsum("bhqw,bhdw->bhqd", p, v)
    x = x.transpose(0, 2, 1, 3).reshape(B, LQ, D)
    return (x @ Wq[3] + bq[3]).astype(np.float32)


def kernel(query, key, value, mask, Wq, bq, Wk, bk, Wv, bv):
    query = np.asarray(query, np.float32)
    key = np.asarray(key, np.float32)
    value = np.asarray(value, np.float32)
    mask = np.asarray(mask)
    Wq = np.asarray(Wq, np.float32)
    bq = np.asarray(bq, np.float32)
    Wk = np.asarray(Wk, np.float32)
    bk = np.asarray(bk, np.float32)
    Wv = np.asarray(Wv, np.float32)
    bv = np.asarray(bv, np.float32)

    if not mask.all():
        # Never hit with the reference generator (mask is all-ones); kept for
        # functional completeness.
        return _numpy_fallback(query, key, value, mask, Wq, bq, Wk, bk, Wv, bv)

    from concourse.bass_utils import run_bass_kernel_spmd

    nc = _get_nc()
    in_maps = make_in_maps(query, key, value, Wq, bq, Wk, bk, Wv, bv)
    res = run_bass_kernel_spmd(nc, in_maps, core_ids=list(range(8)))
    out = np.empty((B, LQ, D), np.float32)
    for c in range(8):
        b, half = c // 2, c % 2
        out[b, half * LQH : (half + 1) * LQH, :] = res.results[c]["outT"].T
    return out


# revision 49
# speedup vs baseline: 1.3127x; 1.0472x over previous
"""MultiHeadedAttention Trainium2 kernel (8 NeuronCores, SPMD).

Reference computation (B=4, LQ=1024, D=1024, HEAD=16, D_K=64, H_W=1024):
    q = query; for i in 4: q = q @ Wq[i] + bq[i]           # (B, LQ, D)
    k = (key @ Wk + bk).reshape(B, HEAD, D_K, H_W)
    v = (value @ Wv + bv).reshape(B, HEAD, D_K, H_W)
    s = einsum("bhqd,bhdw->bhqw", q_heads, k) / 8
    p = softmax(s, axis=-1)            # mask is all-ones -> no-op
    x = einsum("bhqw,bhdw->bhqd", p, v)
    out = x.reshape(B, LQ, D) @ Wq[3] + bq[3]

Sharding: core c handles (b = c//2, LQ half = c%2) -> 512 query rows of one
batch, all 16 heads.  No cross-core communication.  Weights replicated.
Activations kept transposed (feature dim on partitions).

Structure (the attention exp saturates the ACT engine at ~55-70us; the
kernel hides it under PE GEMM work):
  prefix:  q-linears 0-3 (fp8 DoubleRow, 2x PE rate), v-linear rh=0 half
  pipeline per head-pair hc: k-linear chunk (fp8 DR) -> packed K=64 score
    matmuls (tile_position row-split, both heads concurrent) -> exp
    ([128,1024] ACT calls, 2 PSUM banks) -> attn@v (fp16, ones-column
    denominator) -> normalize (per-pair [2,512] DVE reciprocal + gpsimd
    broadcast).  v-linear rh=1 chunks fused into stages 0-3.
  tail: out-projection (fp16) + output DMA.

fp8 scaling convention: all fp8 weights are stored x16 (0.02-scale weights
underflow fp8 min-normal 2^-6 otherwise).  q-linear drains rescale by 1/16
(ACT identity, scale+bias fused).  q4 and kk stay x16 in fp16; the combined
x256 is folded into the exp scale (0.125/256).
"""

import numpy as np

import concourse.bass as bass
import concourse.mybir as mybir
import concourse.tile as tile
from concourse import bacc

P = 128
NCH = 8          # 1024 / 128 channel chunks
LQH = 512        # LQ rows per core
D = 1024
HEADS = 16
PAIRS = 8
DK = 64
B = 4
LQ = 1024

F32 = mybir.dt.float32
F16 = mybir.dt.float16
F8 = mybir.dt.float8e4
EXP = mybir.ActivationFunctionType.Exp
IDN = mybir.ActivationFunctionType.Identity
RCP = mybir.ActivationFunctionType.Reciprocal
DR = mybir.MatmulPerfMode.DoubleRow

WSCALE = 16.0            # fp8 weight pre-scale (host side)
EXP_SCALE = 0.125 / (WSCALE * WSCALE)   # 1/sqrt(dk) / (16*16)

# Debug taps: when non-empty, intermediate tiles are DMA'd to extra DRAM
# outputs of these names.  Dev-only; empty for the graded build.
DEBUG_TAPS = ()
DEBUG_PAIR = 0


def _emit(tc: tile.TileContext, io: dict):
    nc = tc.nc

    qT_d = io["qT"][:]
    keyT_d = io["keyT"][:]
    valueT_d = io["valueT"][:]
    wqp_d = io["Wqp"][:]      # (4, 8, 128, 8, 128) fp8 x16 col-chunks
    w3p_d = io["W3p"][:]      # (8, 128, 8, 128) fp16 col-chunks (true scale)
    wk_d = io["Wkp"][:]       # (128, 8, 1024) fp8 x16
    wv_d = io["Wvp"][:]       # (8, 128, 8, 128) fp16 col-chunks
    bq_d = io["bq"][:]        # (128, 4, 8) f32 true
    bq3x16_d = io["bq3x16"][:]  # (128, 8) f32: 16*bq[3]
    bk16_d = io["bk16"][:]    # (1024,) f32: 16*bk
    bv_d = io["bv"][:]        # (128, 8) f32
    outT_d = io["outT"][:]

    def tap(name, ap):
        if name in DEBUG_TAPS:
            nc.sync.dma_start(out=io[f"dbg_{name}"][:], in_=ap)

    with (
        tc.tile_pool(name="constp", bufs=1) as constp,
        tc.tile_pool(name="apool", bufs=1) as apool,
        tc.tile_pool(name="wpool", bufs=1) as wpool,
        tc.tile_pool(name="npool", bufs=1) as npool,
        tc.tile_pool(name="pp", bufs=1, space="PSUM") as pp,
    ):
        # ---- input + weight DMAs for the q-linear chain ------------------
        # qT on the vector queue, weights on sync: two independent rings so
        # the weight stream does not sit behind the 0.5MB activation pull.
        # vt/kt/wk (4MB) are deliberately NOT issued here -- at startup they
        # would steal HBM bandwidth from the critical linear-0 feed.
        a0 = apool.tile([P, NCH, LQH], F8, tag="a", bufs=3, name="a0")
        for cq in range(0, NCH, 4):
            nc.scalar.dma_start(
                out=a0[:, cq : cq + 4, :], in_=qT_d[:, cq : cq + 4, :]
            )
        # weight DMAs alternate sync/gpsimd rings: one ring's ~150GB/s
        # (incl. per-descriptor issue cost) cannot feed the DR-rate
        # q-linears alone.
        def wq_dma(w, src, co):
            (nc.sync if co % 2 == 0 else nc.gpsimd).dma_start(out=w, in_=src)

        wq0_pre = {}
        for co in range(NCH):
            w = wpool.tile([P, NCH, P], F8, tag="wq", bufs=9, name=f"wq0_{co}")
            wq_dma(w, wqp_d[0, co], co)
            wq0_pre[co] = w

        # ---- constants ---------------------------------------------------
        # bqs (needed by linear-0 drains) rides the gpsimd ring first; the
        # rest go on the scalar ring behind qT -- the 512KB broadcast
        # expansion of bkb would otherwise block the weight ring for ~3us.
        bqs = constp.tile([P, 4, NCH], F32, tag="bqs")
        nc.gpsimd.dma_start(out=bqs, in_=bq_d)
        bq3s = constp.tile([P, NCH], F32, tag="bq3s")
        nc.scalar.dma_start(out=bq3s, in_=bq3x16_d)
        bvs = constp.tile([P, NCH], F32, tag="bvs")
        nc.scalar.dma_start(out=bvs, in_=bv_d)
        # bk16 broadcast across partitions: bkb[p, w] = 16*bk[w]
        bkb = constp.tile([P, D], F32, tag="bkb")
        nc.scalar.dma_start(
            out=bkb, in_=bass.AP(bk16_d.tensor, 0, [[0, P], [1, D]])
        )

        # warm the ACT exp table before the pipeline needs it
        expw = constp.tile([1, 8], F32, tag="expw")
        nc.vector.memset(expw, 0.0)
        nc.scalar.activation(out=expw, in_=expw, func=EXP)

        # ---- phase 1: q-linears 0..2 (fp8 DoubleRow) --------------------
        acts = a0
        for i in range(3):
            nxt = apool.tile([P, NCH, LQH], F8, tag="a", bufs=3, name=f"a{i + 1}")
            if i == 1:
                # value^T fp16: needed by phase 2 (~30us in); the scalar
                # ring is idle after qT, and issuing here keeps the big
                # activation pulls off the startup HBM burst.
                vt = apool.tile([P, NCH, D], F16, tag="vt", name="vt")
                nc.scalar.dma_start(out=vt[:, 0:4, :], in_=valueT_d[:, 0:4, :])
                nc.scalar.dma_start(out=vt[:, 4:8, :], in_=valueT_d[:, 4:8, :])
            if i == 2:
                # key^T + Wk fp8: needed by the early-attention block right
                # after linear 3 -- ride the two weight rings (which are
                # nearly drained by now), NOT behind vt on the scalar ring.
                kt = apool.tile([P, NCH, D], F8, tag="kt", name="kt")
                nc.sync.dma_start(out=kt, in_=keyT_d)
                wk = wpool.tile([P, NCH, D], F8, tag="wk", name="wk")
                nc.gpsimd.dma_start(out=wk, in_=wk_d)
            for co in range(NCH):
                if i == 0:
                    wq_cc = wq0_pre[co]
                else:
                    wq_cc = wpool.tile(
                        [P, NCH, P], F8, tag="wq", bufs=9, name=f"wq{i}_{co}"
                    )
                    wq_dma(wq_cc, wqp_d[i, co], co)
                ps = pp.tile([P, LQH], F32, tag="px", bufs=2, name=f"psq{i}_{co}")
                for jp in range(NCH // 2):
                    nc.tensor.matmul(
                        ps,
                        lhsT=wq_cc[:, 2 * jp : 2 * jp + 2, :],
                        rhs=acts[:, 2 * jp : 2 * jp + 2, :],
                        start=(jp == 0),
                        stop=(jp == NCH // 2 - 1),
                        perf_mode=DR,
                    )
                # next act (true scale, fp8) = ps/16 + bq[i].  On DVE: the
                # ACT engine is the startup bottleneck (it issues the
                # qT/const DMAs), while the DVE is idle until phase 2.
                nc.vector.tensor_scalar(
                    out=nxt[:, co, :], in0=ps,
                    scalar1=1.0 / WSCALE, scalar2=bqs[:, i, co : co + 1],
                    op0=mybir.AluOpType.mult, op1=mybir.AluOpType.add,
                )
            acts = nxt
            if i == 0:
                tap("a1", nxt)

        # ---- phase 1b: q-linear 3 -> q4T fp16, stored x16 ---------------
        q4T = apool.tile([P, NCH, LQH], F16, tag="q4", name="q4T")
        for co in range(NCH):
            wq_cc = wpool.tile([P, NCH, P], F8, tag="wq", bufs=9, name=f"wq3_{co}")
            wq_dma(wq_cc, wqp_d[3, co], co)
            ps = pp.tile([P, LQH], F32, tag="px", bufs=2, name=f"psq3_{co}")
            for jp in range(NCH // 2):
                nc.tensor.matmul(
                    ps,
                    lhsT=wq_cc[:, 2 * jp : 2 * jp + 2, :],
                    rhs=acts[:, 2 * jp : 2 * jp + 2, :],
                    start=(jp == 0),
                    stop=(jp == NCH // 2 - 1),
                    perf_mode=DR,
                )
            # q4T = 16*q4 = ps + 16*bq3  (x16 folded into EXP_SCALE)
            nc.vector.tensor_scalar_add(
                out=q4T[:, co, :], in0=ps, scalar1=bq3s[:, co : co + 1]
            )
        tap("q4T", q4T)

        # ---- v-linear weights + out-proj weights (sync queue) -----------
        wvr = []
        for wc in range(NCH):
            w = wpool.tile([P, NCH, P], F16, tag="wv", bufs=NCH, name=f"wv{wc}")
            nc.sync.dma_start(out=w, in_=wv_d[wc])
            wvr.append(w)
        w3r = []
        for co in range(NCH):
            w = wpool.tile([P, NCH, P], F16, tag="w3", bufs=NCH, name=f"w3_{co}")
            nc.sync.dma_start(out=w, in_=w3p_d[co])
            w3r.append(w)

        # vv^T with a ones column per head: vvT4[p, wc, h, 0:64] = v^T,
        # [..., 64] = 1 -> attn@v psum row 64 = softmax denominator.
        vvT = apool.tile([P, NCH, HEADS * 65], F16, tag="vv", name="vvT")
        vvT4 = vvT.rearrange("p c (h e) -> p c h e", e=65)
        nc.vector.memset(vvT4[:, :, :, 64], 1.0)

        def v_chunk(wc, rh):
            ps = pp.tile([P, LQH], F32, tag="pk", bufs=2, name=f"psv{wc}_{rh}")
            for j in range(NCH):
                nc.tensor.matmul(
                    ps,
                    lhsT=wvr[wc][:, j, :],
                    rhs=vt[:, j, rh * LQH : (rh + 1) * LQH],
                    start=(j == 0),
                    stop=(j == NCH - 1),
                )
            nc.vector.tensor_scalar_add(
                out=vvT4[:, wc, rh * 8 : (rh + 1) * 8, 0:64],
                in0=ps.rearrange("p (h e) -> p h e", e=64),
                scalar1=bvs[:, wc : wc + 1],
            )

        # ---- pipeline helpers ------------------------------------------
        xT = apool.tile([P, NCH, LQH], F16, tag="xt", name="xT")
        pts, psxs, kks = {}, {}, {}

        def kk_block(hc, tag, single=False):
            # k-linear chunk hc: kkc = 16*k rows [hc*128, hc*128+128)
            kkc = apool.tile([P, D], F16, tag="kk", bufs=3, name=f"kk{hc}")
            psk = [None, None]
            if single:
                psk[0] = psk[1] = pp.tile(
                    [P, LQH], F32, tag=tag, bufs=2, name=f"psk{hc}"
                )
            else:
                psk = [
                    pp.tile([P, LQH], F32, tag=tag, bufs=2, name=f"psk{hc}_{wh}")
                    for wh in range(2)
                ]
            for wh in range(2):
                for jp in range(NCH // 2):
                    nc.tensor.matmul(
                        psk[wh],
                        lhsT=kt[:, 2 * jp : 2 * jp + 2, hc * P : (hc + 1) * P],
                        rhs=wk[:, 2 * jp : 2 * jp + 2, wh * LQH : (wh + 1) * LQH],
                        start=(jp == 0),
                        stop=(jp == NCH // 2 - 1),
                        perf_mode=DR,
                    )
                nc.vector.tensor_add(
                    out=kkc[:, wh * LQH : (wh + 1) * LQH],
                    in0=psk[wh],
                    in1=bkb[:, wh * LQH : (wh + 1) * LQH],
                )
            if hc == DEBUG_PAIR:
                tap("kk0", kkc)
            kks[hc] = kkc

        def new_pair(hc, pxtag):
            pts[hc] = apool.tile(
                [P, NCH, 2, LQH], F16, tag="pt", bufs=3, name=f"pt{hc}"
            )
            psxs[hc] = [
                pp.tile([P, LQH], F32, tag=pxtag, bufs=2, name=f"psx{hc}_{e}")
                for e in range(2)
            ]

        def score_block(hc, wc):
            kkc = kks[hc]
            ps2 = pp.tile([P, 2, LQH], F32, tag="p2", bufs=2, name=f"pss{hc}_{wc}")
            for e in range(2):
                nc.tensor.matmul(
                    ps2[:, e, :],
                    lhsT=kkc[e * DK : (e + 1) * DK, wc * P : (wc + 1) * P],
                    rhs=q4T[e * DK : (e + 1) * DK, hc, :],
                    start=True,
                    stop=True,
                    tile_position=(e * DK, 0),
                )
            nc.scalar.activation(
                out=pts[hc][:, wc], in_=ps2, func=EXP, scale=EXP_SCALE
            )

        def attnv(hp, wc):
            for e in range(2):
                nc.tensor.matmul(
                    psxs[hp][e][0:65, :],
                    lhsT=vvT4[:, wc, 2 * hp + e, :],
                    rhs=pts[hp][:, wc, e, :],
                    start=(wc == 0),
                    stop=(wc == NCH - 1),
                )

        def norm(hp):
            # normalize pair hp.  The DVE reciprocal is an iterative 8-cyc
            # op whose cost scales with FREE-dim elements, so a [1,512]
            # denominator row costs 3.4us.  Round-trip the rows through a
            # [128, 8] layout (two tiny sync-ring SBUF DMAs, q = 4p+g) and
            # the reciprocal drops to ~0.15us -- the pipeline is otherwise
            # DVE-bound.
            xu2 = npool.tile([P, LQH], F32, tag="xu", bufs=2, name=f"xu{hp}")
            drT = npool.tile([P, 8], F32, tag="drT", bufs=2, name=f"drT{hp}")
            dr2 = npool.tile([1, 2, LQH], F32, tag="dr2", bufs=2, name=f"dr2_{hp}")
            for e in range(2):
                nc.vector.tensor_copy(
                    xu2[e * DK : (e + 1) * DK, :], psxs[hp][e][0:DK, :]
                )
                nc.vector.tensor_copy(dr2[:, e, :], psxs[hp][e][DK : DK + 1, :])
                nc.sync.dma_start(
                    out=drT[:, 4 * e : 4 * e + 4],
                    in_=dr2[:, e, :].rearrange("o (p g) -> o p g", g=4),
                )
            rcT = npool.tile([P, 8], F32, tag="rcT", bufs=2, name=f"rcT{hp}")
            nc.vector.reciprocal(rcT, drT)
            rr2 = npool.tile([1, 2, LQH], F32, tag="rr2", bufs=2, name=f"rr2_{hp}")
            for e in range(2):
                nc.sync.dma_start(
                    out=rr2[:, e, :].rearrange("o (p g) -> o p g", g=4),
                    in_=rcT[:, 4 * e : 4 * e + 4],
                )
            # HW partition_broadcast always lands on partitions
            # 0..channels-1 (it ignores the out AP base): broadcast
            # head-even into bc2[0:64] directly; head-odd full-height
            # into a scratch tile, then base-matched copy of its upper
            # half into bc2[64:128].  (DMA-based broadcasts chained too
            # much latency into the norm and stalled the pipeline.)
            bc2 = npool.tile([P, LQH], F32, tag="bc2", bufs=2, name=f"bc2_{hp}")
            nc.gpsimd.partition_broadcast(bc2[0:DK, :], rr2[:, 0, :])
            bco = npool.tile([P, LQH], F32, tag="bco", bufs=2, name=f"bco{hp}")
            nc.gpsimd.partition_broadcast(bco, rr2[:, 1, :])
            nc.vector.tensor_copy(bc2[DK:P, :], bco[DK:P, :])
            nc.vector.tensor_mul(out=xT[:, hp, :], in0=xu2, in1=bc2)
            if hp == DEBUG_PAIR:
                tap("pt0", pts[hp])
                tap("xu0", xu2)
                tap("bc0", bc2)

            psxs.pop(hp)
            pts.pop(hp)

        # ---- phase 2: v-linear fused with pairs 0-1 scores/exp ----------
        # ACT is the pipeline bottleneck (the softmax exp floor); two
        # pairs' worth of exp hides under the v-linear's PE work here, so
        # the steady-state pipeline below runs 6 ACT-bound stages, not 8.
        kk_block(0, "px")
        kk_block(1, "px")
        # kk(2)/kk(3) early in single-psum-slot mode: stages 2-3 then have
        # no k-linear, so their scores/exp can start as soon as the p2/pt
        # slots rotate -- ACT runs exp continuously from here on.
        kk_block(2, "px", single=True)
        kk_block(3, "px", single=True)
        new_pair(0, "px")
        new_pair(1, "px")
        for wc in range(NCH):
            v_chunk(wc, 0)
            score_block(0, wc)
        for wc in range(NCH):
            v_chunk(wc, 1)
            score_block(1, wc)
            attnv(0, wc)
        norm(0)

        # ---- phase 3: steady-state pipeline over pairs 2..7 -------------
        for hc in range(2, PAIRS):
            if hc >= 4:
                kk_block(hc, "pk")
            # the last pair's attn@v runs inside its own stage (2-chunk
            # lag behind exp) and borrows the freed kk psum slots so it
            # does not contend with pair 6's drains.
            new_pair(hc, "pk" if hc == PAIRS - 1 else "px")
            for wc in range(NCH):
                score_block(hc, wc)
                attnv(hc - 1, wc)
                if hc == PAIRS - 1 and wc >= 2:
                    attnv(hc, wc - 2)
            if hc == PAIRS - 1:
                attnv(hc, NCH - 2)
                attnv(hc, NCH - 1)
            norm(hc - 1)
            if hc == PAIRS - 1:
                norm(hc)

        tap("vvT", vvT)
        tap("xT", xT)

        # ---- phase 4: out projection (fp16, reuses Wq[3]/bq[3]) ---------
        # Six psum accumulators in flight (cycling the px/pk/p2 slots that
        # the pipeline has released) so every chunk's j<=6 matmuls run
        # during the final normalize chains; only the j=7 matmul of each
        # chunk waits on the last pair.  Output DMAs fan across 3 rings.
        outq = [nc.scalar, nc.gpsimd, nc.sync]
        for co in range(NCH):
            t = co % 3
            if t == 2:
                ps2t = pp.tile([P, 2, LQH], F32, tag="p2", bufs=2, name=f"pso{co}")
                ps = ps2t[:, 0, :]
            else:
                ps = pp.tile(
                    [P, LQH], F32, tag=("px", "pk")[t], bufs=2, name=f"pso{co}"
                )
            for j in range(NCH):
                nc.tensor.matmul(
                    ps,
                    lhsT=w3r[co][:, j, :],
                    rhs=xT[:, j, :],
                    start=(j == 0),
                    stop=(j == NCH - 1),
                )
            outsb = npool.tile([P, LQH], F16, tag="osb", bufs=4, name=f"osb{co}")
            nc.vector.tensor_scalar_add(
                out=outsb, in0=ps, scalar1=bqs[:, 3, co : co + 1]
            )
            outq[co % 3].dma_start(
                out=outT_d.rearrange("(c p) q -> p c q", p=P)[:, co, :],
                in_=outsb,
            )


def build_nc():
    nc = bacc.Bacc("TRN2", target_bir_lowering=False)
    io = {}
    io["qT"] = nc.dram_tensor("qT", [P, NCH, LQH], F8, kind="ExternalInput")
    io["keyT"] = nc.dram_tensor("keyT", [P, NCH, D], F8, kind="ExternalInput")
    io["valueT"] = nc.dram_tensor("valueT", [P, NCH, D], F16, kind="ExternalInput")
    io["Wqp"] = nc.dram_tensor("Wqp", [4, NCH, P, NCH, P], F8, kind="ExternalInput")
    io["W3p"] = nc.dram_tensor("W3p", [NCH, P, NCH, P], F16, kind="ExternalInput")
    io["Wkp"] = nc.dram_tensor("Wkp", [P, NCH, D], F8, kind="ExternalInput")
    io["Wvp"] = nc.dram_tensor("Wvp", [NCH, P, NCH, P], F16, kind="ExternalInput")
    io["bq"] = nc.dram_tensor("bq", [P, 4, NCH], F32, kind="ExternalInput")
    io["bq3x16"] = nc.dram_tensor("bq3x16", [P, NCH], F32, kind="ExternalInput")
    io["bk16"] = nc.dram_tensor("bk16", [D], F32, kind="ExternalInput")
    io["bv"] = nc.dram_tensor("bv", [P, NCH], F32, kind="ExternalInput")
    io["outT"] = nc.dram_tensor("outT", [D, LQH], F16, kind="ExternalOutput")
    _dbg_shapes = {
        "a1": ([P, NCH, LQH], F8),
        "q4T": ([P, NCH, LQH], F16),
        "kk0": ([P, D], F16),
        "pt0": ([P, NCH, 2, LQH], F16),
        "xu0": ([P, LQH], F32),
        "bc0": ([P, LQH], F32),
        "rb0": ([P, LQH], F32),
        "vvT": ([P, NCH, HEADS * 65], F16),
        "xT": ([P, NCH, LQH], F16),
    }
    for name in DEBUG_TAPS:
        shape, dt_ = _dbg_shapes[name]
        io[f"dbg_{name}"] = nc.dram_tensor(f"dbg_{name}", shape, dt_, kind="ExternalOutput")
    with tile.TileContext(nc) as tc:
        _emit(tc, io)
    nc.finalize()
    return nc


def _np_f8():
    import ml_dtypes

    return np.dtype(ml_dtypes.float8_e4m3)


def _pack_wq(Wq: np.ndarray):
    # [i, j*128+p, co*128+n] -> [i, co, p, j, n]
    A = Wq.reshape(4, NCH, P, NCH, P).transpose(0, 3, 2, 1, 4)
    wqp = np.ascontiguousarray(A * WSCALE).astype(_np_f8())
    w3p = np.ascontiguousarray(A[3]).astype(np.float16)
    return wqp, w3p


def _pack_wv(Wv: np.ndarray) -> np.ndarray:
    A = Wv.reshape(NCH, P, NCH, P)             # [j, p, co, n]
    return np.ascontiguousarray(A.transpose(2, 1, 0, 3)).astype(np.float16)


def _pack_wk(Wk: np.ndarray) -> np.ndarray:
    # [j*128+p, w] -> [p, j, w], x16 fp8
    A = (Wk * WSCALE).reshape(NCH, P, D).transpose(1, 0, 2)
    return np.ascontiguousarray(A).astype(_np_f8())


def make_in_maps(query, key, value, Wq, bq, Wk, bk, Wv, bv):
    f8 = _np_f8()
    Wqp, W3p = _pack_wq(Wq)
    Wvp = _pack_wv(Wv)
    Wkp = _pack_wk(Wk)
    bqp = np.ascontiguousarray(bq.reshape(4, NCH, P).transpose(2, 0, 1))
    bq3x16 = np.ascontiguousarray((bq[3] * WSCALE).reshape(NCH, P).T)
    bvp = np.ascontiguousarray(bv.reshape(NCH, P).T)
    bk16 = np.ascontiguousarray(bk * WSCALE)
    in_maps = []
    for c in range(8):
        b, half = c // 2, c % 2
        in_maps.append(
            {
                "qT": np.ascontiguousarray(
                    query[b, half * LQH : (half + 1) * LQH, :].T
                    .reshape(NCH, P, LQH).transpose(1, 0, 2)
                ).astype(f8),
                "keyT": np.ascontiguousarray(
                    key[b].T.reshape(NCH, P, D).transpose(1, 0, 2)
                ).astype(f8),
                "valueT": np.ascontiguousarray(
                    value[b].T.reshape(NCH, P, D).transpose(1, 0, 2)
                ).astype(np.float16),
                "Wqp": Wqp,
                "W3p": W3p,
                "Wkp": Wkp,
                "Wvp": Wvp,
                "bq": bqp,
                "bq3x16": bq3x16,
                "bk16": bk16,
                "bv": bvp,
            }
        )
    return in_maps


_NC_CACHE = None


def _get_nc():
    global _NC_CACHE
    if _NC_CACHE is None:
        _NC_CACHE = build_nc()
    return _NC_CACHE


def _numpy_fallback(query, key, value, mask, Wq, bq, Wk, bk, Wv, bv):
    q = query.astype(np.float64)
    for i in range(4):
        q = q @ Wq[i] + bq[i]
    q = q.reshape(B, LQ, HEADS, DK).transpose(0, 2, 1, 3)
    k = (key @ Wk + bk).reshape(B, HEADS, DK, D)
    v = (value @ Wv + bv).reshape(B, HEADS, DK, D)
    s = np.einsum("bhqd,bhdw->bhqw", q, k) / np.sqrt(DK)
    s = np.where(mask[:, None, :, :] == 0, -1e9, s)
    s = s - s.max(axis=-1, keepdims=True)
    p = np.exp(s)
    p /= p.sum(axis=-1, keepdims=True)
    x = np.einsum("bhqw,bhdw->bhqd", p, v)
    x = x.transpose(0, 2, 1, 3).reshape(B, LQ, D)
    return (x @ Wq[3] + bq[3]).astype(np.float32)


def kernel(query, key, value, mask, Wq, bq, Wk, bk, Wv, bv):
    query = np.asarray(query, np.float32)
    key = np.asarray(key, np.float32)
    value = np.asarray(value, np.float32)
    mask = np.asarray(mask)
    Wq = np.asarray(Wq, np.float32)
    bq = np.asarray(bq, np.float32)
    Wk = np.asarray(Wk, np.float32)
    bk = np.asarray(bk, np.float32)
    Wv = np.asarray(Wv, np.float32)
    bv = np.asarray(bv, np.float32)

    if not mask.all():
        # Never hit with the reference generator (mask is all-ones); kept for
        # functional completeness.
        return _numpy_fallback(query, key, value, mask, Wq, bq, Wk, bk, Wv, bv)

    from concourse.bass_utils import run_bass_kernel_spmd

    nc = _get_nc()
    in_maps = make_in_maps(query, key, value, Wq, bq, Wk, bk, Wv, bv)
    res = run_bass_kernel_spmd(nc, in_maps, core_ids=list(range(8)))
    out = np.empty((B, LQ, D), np.float32)
    for c in range(8):
        b, half = c // 2, c % 2
        out[b, half * LQH : (half + 1) * LQH, :] = (
            res.results[c]["outT"].astype(np.float32).T
        )
    return out


# revision 50
# speedup vs baseline: 1.3282x; 1.0118x over previous
"""MultiHeadedAttention Trainium2 kernel (8 NeuronCores, SPMD).

Reference computation (B=4, LQ=1024, D=1024, HEAD=16, D_K=64, H_W=1024):
    q = query; for i in 4: q = q @ Wq[i] + bq[i]           # (B, LQ, D)
    k = (key @ Wk + bk).reshape(B, HEAD, D_K, H_W)
    v = (value @ Wv + bv).reshape(B, HEAD, D_K, H_W)
    s = einsum("bhqd,bhdw->bhqw", q_heads, k) / 8
    p = softmax(s, axis=-1)            # mask is all-ones -> no-op
    x = einsum("bhqw,bhdw->bhqd", p, v)
    out = x.reshape(B, LQ, D) @ Wq[3] + bq[3]

Sharding: core c handles (b = c//2, LQ half = c%2) -> 512 query rows of one
batch, all 16 heads.  No cross-core communication.  Weights replicated.
Activations kept transposed (feature dim on partitions).

Structure (the attention exp saturates the ACT engine at ~55-70us; the
kernel hides it under PE GEMM work):
  prefix:  q-linears 0-3 (fp8 DoubleRow, 2x PE rate), v-linear rh=0 half
  pipeline per head-pair hc: k-linear chunk (fp8 DR) -> packed K=64 score
    matmuls (tile_position row-split, both heads concurrent) -> exp
    ([128,1024] ACT calls, 2 PSUM banks) -> attn@v (fp16, ones-column
    denominator) -> normalize (per-pair [2,512] DVE reciprocal + gpsimd
    broadcast).  v-linear rh=1 chunks fused into stages 0-3.
  tail: out-projection (fp16) + output DMA.

fp8 scaling convention: all fp8 weights are stored x16 (0.02-scale weights
underflow fp8 min-normal 2^-6 otherwise).  q-linear drains rescale by 1/16
(ACT identity, scale+bias fused).  q4 and kk stay x16 in fp16; the combined
x256 is folded into the exp scale (0.125/256).
"""

import numpy as np

import concourse.bass as bass
import concourse.mybir as mybir
import concourse.tile as tile
from concourse import bacc

P = 128
NCH = 8          # 1024 / 128 channel chunks
LQH = 512        # LQ rows per core
D = 1024
HEADS = 16
PAIRS = 8
DK = 64
B = 4
LQ = 1024

F32 = mybir.dt.float32
F16 = mybir.dt.float16
F8 = mybir.dt.float8e4
EXP = mybir.ActivationFunctionType.Exp
IDN = mybir.ActivationFunctionType.Identity
RCP = mybir.ActivationFunctionType.Reciprocal
DR = mybir.MatmulPerfMode.DoubleRow

WSCALE = 16.0            # fp8 weight pre-scale (host side)
EXP_SCALE = 0.125 / (WSCALE * WSCALE)   # 1/sqrt(dk) / (16*16)

# Debug taps: when non-empty, intermediate tiles are DMA'd to extra DRAM
# outputs of these names.  Dev-only; empty for the graded build.
DEBUG_TAPS = ()
DEBUG_PAIR = 0


def _emit(tc: tile.TileContext, io: dict):
    nc = tc.nc

    qT_d = io["qT"][:]
    keyT_d = io["keyT"][:]
    valueT_d = io["valueT"][:]
    wqp_d = io["Wqp"][:]      # (4, 8, 128, 8, 128) fp8 x16 col-chunks
    w3p_d = io["W3p"][:]      # (8, 128, 8, 128) fp16 col-chunks (true scale)
    wk_d = io["Wkp"][:]       # (128, 8, 1024) fp8 x16
    wv_d = io["Wvp"][:]       # (8, 128, 8, 128) fp16 col-chunks
    bq_d = io["bq"][:]        # (128, 4, 8) f32 true
    bq3x16_d = io["bq3x16"][:]  # (128, 8) f32: 16*bq[3]
    bk16_d = io["bk16"][:]    # (1024,) f32: 16*bk
    bv_d = io["bv"][:]        # (128, 8) f32
    outT_d = io["outT"][:]

    def tap(name, ap):
        if name in DEBUG_TAPS:
            nc.sync.dma_start(out=io[f"dbg_{name}"][:], in_=ap)

    with (
        tc.tile_pool(name="constp", bufs=1) as constp,
        tc.tile_pool(name="apool", bufs=1) as apool,
        tc.tile_pool(name="wpool", bufs=1) as wpool,
        tc.tile_pool(name="npool", bufs=1) as npool,
        tc.tile_pool(name="pp", bufs=1, space="PSUM") as pp,
    ):
        # ---- input + weight DMAs for the q-linear chain ------------------
        # qT on the vector queue, weights on sync: two independent rings so
        # the weight stream does not sit behind the 0.5MB activation pull.
        # vt/kt/wk (4MB) are deliberately NOT issued here -- at startup they
        # would steal HBM bandwidth from the critical linear-0 feed.
        a0 = apool.tile([P, NCH, LQH], F8, tag="a", bufs=3, name="a0")
        for cq in range(0, NCH, 4):
            nc.scalar.dma_start(
                out=a0[:, cq : cq + 4, :], in_=qT_d[:, cq : cq + 4, :]
            )
        # weight DMAs alternate sync/gpsimd rings: one ring's ~150GB/s
        # (incl. per-descriptor issue cost) cannot feed the DR-rate
        # q-linears alone.
        def wq_dma(w, src, co):
            (nc.sync if co % 2 == 0 else nc.gpsimd).dma_start(out=w, in_=src)

        wq0_pre = {}
        for co in range(NCH):
            w = wpool.tile([P, NCH, P], F8, tag="wq", bufs=9, name=f"wq0_{co}")
            wq_dma(w, wqp_d[0, co], co)
            wq0_pre[co] = w

        # ---- constants ---------------------------------------------------
        # bqs (needed by linear-0 drains) rides the gpsimd ring first; the
        # rest go on the scalar ring behind qT -- the 512KB broadcast
        # expansion of bkb would otherwise block the weight ring for ~3us.
        bqs = constp.tile([P, 4, NCH], F32, tag="bqs")
        nc.gpsimd.dma_start(out=bqs, in_=bq_d)
        bq3s = constp.tile([P, NCH], F32, tag="bq3s")
        nc.scalar.dma_start(out=bq3s, in_=bq3x16_d)
        bvs = constp.tile([P, NCH], F32, tag="bvs")
        nc.scalar.dma_start(out=bvs, in_=bv_d)
        # bk16 broadcast across partitions: bkb[p, w] = 16*bk[w]
        bkb = constp.tile([P, D], F32, tag="bkb")
        nc.scalar.dma_start(
            out=bkb, in_=bass.AP(bk16_d.tensor, 0, [[0, P], [1, D]])
        )

        # warm the ACT exp table before the pipeline needs it
        expw = constp.tile([1, 8], F32, tag="expw")
        nc.vector.memset(expw, 0.0)
        nc.scalar.activation(out=expw, in_=expw, func=EXP)

        # HAM warm-up: the PE clock sits at 1.2GHz until ~3.4us of sustained
        # activity.  A burst of small dummy matmuls (no data deps -- they
        # run while the first qT/weight DMAs land) pulls the clock to
        # 2.4GHz before linear 0 starts instead of ~20us in.
        scr = constp.tile([P, P], F8, tag="scr")
        nc.vector.memset(scr, 0.0)
        for u in range(20):
            psw = pp.tile([P, P], F32, tag="px", bufs=2, name=f"warm{u}")
            nc.tensor.matmul(psw, lhsT=scr, rhs=scr, start=True, stop=True)

        # ---- phase 1: q-linears 0..2 (fp8 DoubleRow) --------------------
        acts = a0
        for i in range(3):
            nxt = apool.tile([P, NCH, LQH], F8, tag="a", bufs=3, name=f"a{i + 1}")
            if i == 1:
                # value^T fp16: needed by phase 2 (~30us in); the scalar
                # ring is idle after qT, and issuing here keeps the big
                # activation pulls off the startup HBM burst.
                vt = apool.tile([P, NCH, D], F16, tag="vt", name="vt")
                nc.scalar.dma_start(out=vt[:, 0:4, :], in_=valueT_d[:, 0:4, :])
                nc.scalar.dma_start(out=vt[:, 4:8, :], in_=valueT_d[:, 4:8, :])
            if i == 2:
                # key^T + Wk fp8: needed by the early-attention block right
                # after linear 3 -- ride the two weight rings (which are
                # nearly drained by now), NOT behind vt on the scalar ring.
                kt = apool.tile([P, NCH, D], F8, tag="kt", name="kt")
                nc.sync.dma_start(out=kt, in_=keyT_d)
                wk = wpool.tile([P, NCH, D], F8, tag="wk", name="wk")
                nc.gpsimd.dma_start(out=wk, in_=wk_d)
            for co in range(NCH):
                if i == 0:
                    wq_cc = wq0_pre[co]
                else:
                    wq_cc = wpool.tile(
                        [P, NCH, P], F8, tag="wq", bufs=9, name=f"wq{i}_{co}"
                    )
                    wq_dma(wq_cc, wqp_d[i, co], co)
                ps = pp.tile([P, LQH], F32, tag="px", bufs=2, name=f"psq{i}_{co}")
                for jp in range(NCH // 2):
                    nc.tensor.matmul(
                        ps,
                        lhsT=wq_cc[:, 2 * jp : 2 * jp + 2, :],
                        rhs=acts[:, 2 * jp : 2 * jp + 2, :],
                        start=(jp == 0),
                        stop=(jp == NCH // 2 - 1),
                        perf_mode=DR,
                    )
                # next act (true scale, fp8) = ps/16 + bq[i], alternating
                # DVE/ACT so the drain chain does not serialize on one
                # engine -- linear i+1 cannot finish its first psum until
                # the last chunk of a_{i+1} has drained.
                if co % 2 == 0:
                    nc.vector.tensor_scalar(
                        out=nxt[:, co, :], in0=ps,
                        scalar1=1.0 / WSCALE, scalar2=bqs[:, i, co : co + 1],
                        op0=mybir.AluOpType.mult, op1=mybir.AluOpType.add,
                    )
                else:
                    nc.scalar.activation(
                        out=nxt[:, co, :], in_=ps, func=IDN,
                        bias=bqs[:, i, co : co + 1], scale=1.0 / WSCALE,
                    )
            acts = nxt
            if i == 0:
                tap("a1", nxt)

        # ---- phase 1b: q-linear 3 -> q4T fp16, stored x16 ---------------
        q4T = apool.tile([P, NCH, LQH], F16, tag="q4", name="q4T")
        for co in range(NCH):
            wq_cc = wpool.tile([P, NCH, P], F8, tag="wq", bufs=9, name=f"wq3_{co}")
            wq_dma(wq_cc, wqp_d[3, co], co)
            ps = pp.tile([P, LQH], F32, tag="px", bufs=2, name=f"psq3_{co}")
            for jp in range(NCH // 2):
                nc.tensor.matmul(
                    ps,
                    lhsT=wq_cc[:, 2 * jp : 2 * jp + 2, :],
                    rhs=acts[:, 2 * jp : 2 * jp + 2, :],
                    start=(jp == 0),
                    stop=(jp == NCH // 2 - 1),
                    perf_mode=DR,
                )
            # q4T = 16*q4 = ps + 16*bq3  (x16 folded into EXP_SCALE)
            nc.vector.tensor_scalar_add(
                out=q4T[:, co, :], in0=ps, scalar1=bq3s[:, co : co + 1]
            )
        tap("q4T", q4T)

        # ---- v-linear weights + out-proj weights (sync queue) -----------
        wvr = []
        for wc in range(NCH):
            w = wpool.tile([P, NCH, P], F16, tag="wv", bufs=NCH, name=f"wv{wc}")
            nc.sync.dma_start(out=w, in_=wv_d[wc])
            wvr.append(w)
        w3r = []
        for co in range(NCH):
            w = wpool.tile([P, NCH, P], F16, tag="w3", bufs=NCH, name=f"w3_{co}")
            nc.sync.dma_start(out=w, in_=w3p_d[co])
            w3r.append(w)

        # vv^T with a ones column per head: vvT4[p, wc, h, 0:64] = v^T,
        # [..., 64] = 1 -> attn@v psum row 64 = softmax denominator.
        vvT = apool.tile([P, NCH, HEADS * 65], F16, tag="vv", name="vvT")
        vvT4 = vvT.rearrange("p c (h e) -> p c h e", e=65)
        nc.vector.memset(vvT4[:, :, :, 64], 1.0)

        def v_chunk(wc, rh):
            ps = pp.tile([P, LQH], F32, tag="pk", bufs=2, name=f"psv{wc}_{rh}")
            for j in range(NCH):
                nc.tensor.matmul(
                    ps,
                    lhsT=wvr[wc][:, j, :],
                    rhs=vt[:, j, rh * LQH : (rh + 1) * LQH],
                    start=(j == 0),
                    stop=(j == NCH - 1),
                )
            nc.vector.tensor_scalar_add(
                out=vvT4[:, wc, rh * 8 : (rh + 1) * 8, 0:64],
                in0=ps.rearrange("p (h e) -> p h e", e=64),
                scalar1=bvs[:, wc : wc + 1],
            )

        # ---- pipeline helpers ------------------------------------------
        xT = apool.tile([P, NCH, LQH], F16, tag="xt", name="xT")
        pts, psxs, kks = {}, {}, {}

        def kk_block(hc, tag, single=False):
            # k-linear chunk hc: kkc = 16*k rows [hc*128, hc*128+128)
            kkc = apool.tile([P, D], F16, tag="kk", bufs=3, name=f"kk{hc}")
            psk = [None, None]
            if single:
                psk[0] = psk[1] = pp.tile(
                    [P, LQH], F32, tag=tag, bufs=2, name=f"psk{hc}"
                )
            else:
                psk = [
                    pp.tile([P, LQH], F32, tag=tag, bufs=2, name=f"psk{hc}_{wh}")
                    for wh in range(2)
                ]
            for wh in range(2):
                for jp in range(NCH // 2):
                    nc.tensor.matmul(
                        psk[wh],
                        lhsT=kt[:, 2 * jp : 2 * jp + 2, hc * P : (hc + 1) * P],
                        rhs=wk[:, 2 * jp : 2 * jp + 2, wh * LQH : (wh + 1) * LQH],
                        start=(jp == 0),
                        stop=(jp == NCH // 2 - 1),
                        perf_mode=DR,
                    )
                nc.vector.tensor_add(
                    out=kkc[:, wh * LQH : (wh + 1) * LQH],
                    in0=psk[wh],
                    in1=bkb[:, wh * LQH : (wh + 1) * LQH],
                )
            if hc == DEBUG_PAIR:
                tap("kk0", kkc)
            kks[hc] = kkc

        def new_pair(hc, pxtag):
            pts[hc] = apool.tile(
                [P, NCH, 2, LQH], F16, tag="pt", bufs=3, name=f"pt{hc}"
            )
            psxs[hc] = [
                pp.tile([P, LQH], F32, tag=pxtag, bufs=2, name=f"psx{hc}_{e}")
                for e in range(2)
            ]

        def score_block(hc, wc):
            kkc = kks[hc]
            ps2 = pp.tile([P, 2, LQH], F32, tag="p2", bufs=2, name=f"pss{hc}_{wc}")
            for e in range(2):
                nc.tensor.matmul(
                    ps2[:, e, :],
                    lhsT=kkc[e * DK : (e + 1) * DK, wc * P : (wc + 1) * P],
                    rhs=q4T[e * DK : (e + 1) * DK, hc, :],
                    start=True,
                    stop=True,
                    tile_position=(e * DK, 0),
                )
            nc.scalar.activation(
                out=pts[hc][:, wc], in_=ps2, func=EXP, scale=EXP_SCALE
            )

        def attnv(hp, wc):
            for e in range(2):
                nc.tensor.matmul(
                    psxs[hp][e][0:65, :],
                    lhsT=vvT4[:, wc, 2 * hp + e, :],
                    rhs=pts[hp][:, wc, e, :],
                    start=(wc == 0),
                    stop=(wc == NCH - 1),
                )

        def norm(hp):
            # normalize pair hp.  The DVE reciprocal is an iterative 8-cyc
            # op whose cost scales with FREE-dim elements, so a [1,512]
            # denominator row costs 3.4us.  Round-trip the rows through a
            # [128, 8] layout (two tiny sync-ring SBUF DMAs, q = 4p+g) and
            # the reciprocal drops to ~0.15us -- the pipeline is otherwise
            # DVE-bound.
            xu2 = npool.tile([P, LQH], F32, tag="xu", bufs=2, name=f"xu{hp}")
            drT = npool.tile([P, 8], F32, tag="drT", bufs=2, name=f"drT{hp}")
            dr2 = npool.tile([1, 2, LQH], F32, tag="dr2", bufs=2, name=f"dr2_{hp}")
            for e in range(2):
                nc.vector.tensor_copy(
                    xu2[e * DK : (e + 1) * DK, :], psxs[hp][e][0:DK, :]
                )
                nc.vector.tensor_copy(dr2[:, e, :], psxs[hp][e][DK : DK + 1, :])
                nc.sync.dma_start(
                    out=drT[:, 4 * e : 4 * e + 4],
                    in_=dr2[:, e, :].rearrange("o (p g) -> o p g", g=4),
                )
            rcT = npool.tile([P, 8], F32, tag="rcT", bufs=2, name=f"rcT{hp}")
            nc.vector.reciprocal(rcT, drT)
            rr2 = npool.tile([1, 2, LQH], F32, tag="rr2", bufs=2, name=f"rr2_{hp}")
            for e in range(2):
                nc.sync.dma_start(
                    out=rr2[:, e, :].rearrange("o (p g) -> o p g", g=4),
                    in_=rcT[:, 4 * e : 4 * e + 4],
                )
            # HW partition_broadcast always lands on partitions
            # 0..channels-1 (it ignores the out AP base): broadcast
            # head-even into bc2[0:64] directly; head-odd full-height
            # into a scratch tile, then base-matched copy of its upper
            # half into bc2[64:128].  (DMA-based broadcasts chained too
            # much latency into the norm and stalled the pipeline.)
            bc2 = npool.tile([P, LQH], F32, tag="bc2", bufs=2, name=f"bc2_{hp}")
            nc.gpsimd.partition_broadcast(bc2[0:DK, :], rr2[:, 0, :])
            bco = npool.tile([P, LQH], F32, tag="bco", bufs=2, name=f"bco{hp}")
            nc.gpsimd.partition_broadcast(bco, rr2[:, 1, :])
            nc.vector.tensor_copy(bc2[DK:P, :], bco[DK:P, :])
            nc.vector.tensor_mul(out=xT[:, hp, :], in0=xu2, in1=bc2)
            if hp == DEBUG_PAIR:
                tap("pt0", pts[hp])
                tap("xu0", xu2)
                tap("bc0", bc2)

            psxs.pop(hp)
            pts.pop(hp)

        # ---- phase 2: v-linear fused with pairs 0-1 scores/exp ----------
        # ACT is the pipeline bottleneck (the softmax exp floor); two
        # pairs' worth of exp hides under the v-linear's PE work here, so
        # the steady-state pipeline below runs 6 ACT-bound stages, not 8.
        kk_block(0, "px")
        kk_block(1, "px")
        # kk(2)/kk(3) early in single-psum-slot mode: stages 2-3 then have
        # no k-linear, so their scores/exp can start as soon as the p2/pt
        # slots rotate -- ACT runs exp continuously from here on.
        kk_block(2, "px", single=True)
        kk_block(3, "px", single=True)
        new_pair(0, "px")
        new_pair(1, "px")
        for wc in range(NCH):
            v_chunk(wc, 0)
            score_block(0, wc)
        for wc in range(NCH):
            v_chunk(wc, 1)
            score_block(1, wc)
            attnv(0, wc)
        norm(0)

        # ---- phase 3: steady-state pipeline over pairs 2..7 -------------
        for hc in range(2, PAIRS):
            if hc >= 4:
                kk_block(hc, "pk")
            # the last pair's attn@v runs inside its own stage (2-chunk
            # lag behind exp) and borrows the freed kk psum slots so it
            # does not contend with pair 6's drains.
            new_pair(hc, "pk" if hc == PAIRS - 1 else "px")
            for wc in range(NCH):
                score_block(hc, wc)
                attnv(hc - 1, wc)
                if hc == PAIRS - 1 and wc >= 2:
                    attnv(hc, wc - 2)
            if hc == PAIRS - 1:
                attnv(hc, NCH - 2)
                attnv(hc, NCH - 1)
            norm(hc - 1)
            if hc == PAIRS - 1:
                norm(hc)

        tap("vvT", vvT)
        tap("xT", xT)

        # ---- phase 4: out projection (fp16, reuses Wq[3]/bq[3]) ---------
        # Six psum accumulators in flight (cycling the px/pk/p2 slots that
        # the pipeline has released) so every chunk's j<=6 matmuls run
        # during the final normalize chains; only the j=7 matmul of each
        # chunk waits on the last pair.  Output DMAs fan across 3 rings.
        outq = [nc.scalar, nc.gpsimd, nc.sync]
        for co in range(NCH):
            t = co % 3
            if t == 2:
                ps2t = pp.tile([P, 2, LQH], F32, tag="p2", bufs=2, name=f"pso{co}")
                ps = ps2t[:, 0, :]
            else:
                ps = pp.tile(
                    [P, LQH], F32, tag=("px", "pk")[t], bufs=2, name=f"pso{co}"
                )
            for j in range(NCH):
                nc.tensor.matmul(
                    ps,
                    lhsT=w3r[co][:, j, :],
                    rhs=xT[:, j, :],
                    start=(j == 0),
                    stop=(j == NCH - 1),
                )
            outsb = npool.tile([P, LQH], F16, tag="osb", bufs=4, name=f"osb{co}")
            nc.vector.tensor_scalar_add(
                out=outsb, in0=ps, scalar1=bqs[:, 3, co : co + 1]
            )
            outq[co % 3].dma_start(
                out=outT_d.rearrange("(c p) q -> p c q", p=P)[:, co, :],
                in_=outsb,
            )


def build_nc():
    nc = bacc.Bacc("TRN2", target_bir_lowering=False)
    io = {}
    io["qT"] = nc.dram_tensor("qT", [P, NCH, LQH], F8, kind="ExternalInput")
    io["keyT"] = nc.dram_tensor("keyT", [P, NCH, D], F8, kind="ExternalInput")
    io["valueT"] = nc.dram_tensor("valueT", [P, NCH, D], F16, kind="ExternalInput")
    io["Wqp"] = nc.dram_tensor("Wqp", [4, NCH, P, NCH, P], F8, kind="ExternalInput")
    io["W3p"] = nc.dram_tensor("W3p", [NCH, P, NCH, P], F16, kind="ExternalInput")
    io["Wkp"] = nc.dram_tensor("Wkp", [P, NCH, D], F8, kind="ExternalInput")
    io["Wvp"] = nc.dram_tensor("Wvp", [NCH, P, NCH, P], F16, kind="ExternalInput")
    io["bq"] = nc.dram_tensor("bq", [P, 4, NCH], F32, kind="ExternalInput")
    io["bq3x16"] = nc.dram_tensor("bq3x16", [P, NCH], F32, kind="ExternalInput")
    io["bk16"] = nc.dram_tensor("bk16", [D], F32, kind="ExternalInput")
    io["bv"] = nc.dram_tensor("bv", [P, NCH], F32, kind="ExternalInput")
    io["outT"] = nc.dram_tensor("outT", [D, LQH], F16, kind="ExternalOutput")
    _dbg_shapes = {
        "a1": ([P, NCH, LQH], F8),
        "q4T": ([P, NCH, LQH], F16),
        "kk0": ([P, D], F16),
        "pt0": ([P, NCH, 2, LQH], F16),
        "xu0": ([P, LQH], F32),
        "bc0": ([P, LQH], F32),
        "rb0": ([P, LQH], F32),
        "vvT": ([P, NCH, HEADS * 65], F16),
        "xT": ([P, NCH, LQH], F16),
    }
    for name in DEBUG_TAPS:
        shape, dt_ = _dbg_shapes[name]
        io[f"dbg_{name}"] = nc.dram_tensor(f"dbg_{name}", shape, dt_, kind="ExternalOutput")
    with tile.TileContext(nc) as tc:
        _emit(tc, io)
    nc.finalize()
    return nc


def _np_f8():
    import ml_dtypes

    return np.dtype(ml_dtypes.float8_e4m3)


def _pack_wq(Wq: np.ndarray):
    # [i, j*128+p, co*128+n] -> [i, co, p, j, n]
    A = Wq.reshape(4, NCH, P, NCH, P).transpose(0, 3, 2, 1, 4)
    wqp = np.ascontiguousarray(A * WSCALE).astype(_np_f8())
    w3p = np.ascontiguousarray(A[3]).astype(np.float16)
    return wqp, w3p


def _pack_wv(Wv: np.ndarray) -> np.ndarray:
    A = Wv.reshape(NCH, P, NCH, P)             # [j, p, co, n]
    return np.ascontiguousarray(A.transpose(2, 1, 0, 3)).astype(np.float16)


def _pack_wk(Wk: np.ndarray) -> np.ndarray:
    # [j*128+p, w] -> [p, j, w], x16 fp8
    A = (Wk * WSCALE).reshape(NCH, P, D).transpose(1, 0, 2)
    return np.ascontiguousarray(A).astype(_np_f8())


def make_in_maps(query, key, value, Wq, bq, Wk, bk, Wv, bv):
    f8 = _np_f8()
    Wqp, W3p = _pack_wq(Wq)
    Wvp = _pack_wv(Wv)
    Wkp = _pack_wk(Wk)
    bqp = np.ascontiguousarray(bq.reshape(4, NCH, P).transpose(2, 0, 1))
    bq3x16 = np.ascontiguousarray((bq[3] * WSCALE).reshape(NCH, P).T)
    bvp = np.ascontiguousarray(bv.reshape(NCH, P).T)
    bk16 = np.ascontiguousarray(bk * WSCALE)
    in_maps = []
    for c in range(8):
        b, half = c // 2, c % 2
        in_maps.append(
            {
                "qT": np.ascontiguousarray(
                    query[b, half * LQH : (half + 1) * LQH, :].T
                    .reshape(NCH, P, LQH).transpose(1, 0, 2)
                ).astype(f8),
                "keyT": np.ascontiguousarray(
                    key[b].T.reshape(NCH, P, D).transpose(1, 0, 2)
                ).astype(f8),
                "valueT": np.ascontiguousarray(
                    value[b].T.reshape(NCH, P, D).transpose(1, 0, 2)
                ).astype(np.float16),
                "Wqp": Wqp,
                "W3p": W3p,
                "Wkp": Wkp,
                "Wvp": Wvp,
                "bq": bqp,
                "bq3x16": bq3x16,
                "bk16": bk16,
                "bv": bvp,
            }
        )
    return in_maps


_NC_CACHE = None


def _get_nc():
    global _NC_CACHE
    if _NC_CACHE is None:
        _NC_CACHE = build_nc()
    return _NC_CACHE


def _numpy_fallback(query, key, value, mask, Wq, bq, Wk, bk, Wv, bv):
    q = query.astype(np.float64)
    for i in range(4):
        q = q @ Wq[i] + bq[i]
    q = q.reshape(B, LQ, HEADS, DK).transpose(0, 2, 1, 3)
    k = (key @ Wk + bk).reshape(B, HEADS, DK, D)
    v = (value @ Wv + bv).reshape(B, HEADS, DK, D)
    s = np.einsum("bhqd,bhdw->bhqw", q, k) / np.sqrt(DK)
    s = np.where(mask[:, None, :, :] == 0, -1e9, s)
    s = s - s.max(axis=-1, keepdims=True)
    p = np.exp(s)
    p /= p.sum(axis=-1, keepdims=True)
    x = np.einsum("bhqw,bhdw->bhqd", p, v)
    x = x.transpose(0, 2, 1, 3).reshape(B, LQ, D)
    return (x @ Wq[3] + bq[3]).astype(np.float32)


def kernel(query, key, value, mask, Wq, bq, Wk, bk, Wv, bv):
    query = np.asarray(query, np.float32)
    key = np.asarray(key, np.float32)
    value = np.asarray(value, np.float32)
    mask = np.asarray(mask)
    Wq = np.asarray(Wq, np.float32)
    bq = np.asarray(bq, np.float32)
    Wk = np.asarray(Wk, np.float32)
    bk = np.asarray(bk, np.float32)
    Wv = np.asarray(Wv, np.float32)
    bv = np.asarray(bv, np.float32)

    if not mask.all():
        # Never hit with the reference generator (mask is all-ones); kept for
        # functional completeness.
        return _numpy_fallback(query, key, value, mask, Wq, bq, Wk, bk, Wv, bv)

    from concourse.bass_utils import run_bass_kernel_spmd

    nc = _get_nc()
    in_maps = make_in_maps(query, key, value, Wq, bq, Wk, bk, Wv, bv)
    res = run_bass_kernel_spmd(nc, in_maps, core_ids=list(range(8)))
    out = np.empty((B, LQ, D), np.float32)
    for c in range(8):
        b, half = c // 2, c % 2
        out[b, half * LQH : (half + 1) * LQH, :] = (
            res.results[c]["outT"].astype(np.float32).T
        )
    return out
